# revision 11
# baseline (speedup 1.0000x reference)
"""GATConv (multi-head graph attention) on 8 Trainium2 NeuronCores.

kernel(**inputs) takes the FULL numpy inputs and returns the FULL
[50000, 256] float32 output.  All floating-point math runs on-device in a
Bass/Tile kernel; the host only does index bookkeeping (edge sorting,
gather-index tables, padding) and dtype conversion of inputs.

Distribution: nodes are block-partitioned across the 8 cores.  Per core:
phase 0 projects its node slice (x @ [W | W@A] bf16 matmuls) producing h
and the attention dot-products s; phase 1 computes softmax denominators
for its source nodes; phase 2 aggregates messages at its destination
nodes.  Gathers are batched across node blocks (few large dma_gather
calls instead of many small ones), one-hot selector tiles are built
on-chip from 2-byte rel indices, and the large h AllGather overlaps
phase 1 (the per-node reciprocals are delivered later via a tiny compact
AllGather plus a strided DRAM->DRAM write into the gathered-row slots).
"""

import sys

sys.path.insert(0, "/opt/trn_rl_repo")

import numpy as np

N_NODES = 50000
N_EDGES = 800000
IN_DIM = 512
HEADS = 4
OUT_DIM = 64
F = HEADS * OUT_DIM  # 256
N_CORES = 8
HALF = 32768  # int16 gather index range split
HROW = 256  # H table bf16 elems/row: h only -> 512B rows
SROW = 64  # SR table f32 elems/row: 4 s_src | 4 s_dst | 4 recip | pad -> 256B

CB1 = 5  # blocks per chunk, phase 1
CB2 = 3  # blocks per chunk, phase 2
MAXT1 = 48  # max tiles per dma_gather call, phase 1 (256B rows)
MAXT2 = 32  # max tiles per dma_gather call, phase 2 (768B rows)


def _ceil_div(a, b):
    return (a + b - 1) // b


def _wrap16(arr_i16):
    """dma_gather idx layout: position i -> [i % 16, i // 16], x8 core groups."""
    n = arr_i16.shape[0]
    assert n % 16 == 0
    w = arr_i16.reshape(n // 16, 16).T
    return np.ascontiguousarray(np.tile(w, (8, 1)))


def _chunk_plan(nblk, cb_max):
    chunks = []
    c0 = 0
    while c0 < nblk:
        cb = min(cb_max, nblk - c0)
        chunks.append((c0, cb))
        c0 += cb
    return chunks


def _build_phase_meta(key, other, n_cores, nblk, cb_max):
    """Per-core gather indices + rel tables for one edge pass.

    Edges are grouped by key-node block (128 nodes).  Chunk layout: for a
    chunk of cb blocks, the tile stream is [cb*T_lo low-half tiles |
    cb*T_hi high-half tiles]; block j's low tiles start at j*T_lo, its
    high tiles at cb*T_lo + j*T_hi.  (low/high = gathered node < HALF.)

    Returns (T_lo, T_hi, chunks, per_core) with per_core[c] =
      (gidx [128, ntiles*8] i16  - remote gather idx into half table,
       oidx [128, ntiles*8] i16  - own-node local idx into S_ownE,
       rel  [128, ntiles]  bf16  - key % 128 per lane, 128.0 for pads).
    """
    import ml_dtypes

    rpc = nblk * 128
    gblk = (key >> 7).astype(np.int64)
    hi = (other >= HALF).astype(np.int64)
    order = np.lexsort((other, hi, gblk))
    kb = key[order]
    ob = other[order]
    hb = hi[order]
    gb = gblk[order]

    nrun = n_cores * nblk * 2
    run = gb * 2 + hb
    counts = np.bincount(run, minlength=nrun)
    T_lo = max(1, int(np.max(_ceil_div(counts[0::2], 128))))
    T_hi = int(np.max(_ceil_div(counts[1::2], 128)))
    T = T_lo + T_hi

    chunks = _chunk_plan(nblk, cb_max)
    ntiles = nblk * T
    tb = np.zeros(len(chunks) + 1, np.int64)
    for i, (_, cb_) in enumerate(chunks):
        tb[i + 1] = tb[i] + cb_ * T
    assert tb[-1] == ntiles

    starts = np.zeros(nrun, np.int64)
    np.cumsum(counts[:-1], out=starts[1:])
    rank = np.arange(len(kb), dtype=np.int64) - starts[run]
    blocal = gb % nblk
    core = gb // nblk
    chunk_id = blocal // cb_max
    c0_arr = chunk_id * cb_max
    cb_arr = np.minimum(cb_max, nblk - c0_arr)
    tic = np.where(
        hb == 0,
        (blocal - c0_arr) * T_lo + rank // 128,
        cb_arr * T_lo + (blocal - c0_arr) * T_hi + rank // 128,
    )
    tile = tb[chunk_id] + tic
    pos = (core * ntiles + tile) * 128 + rank % 128

    total = n_cores * ntiles * 128
    gidx = np.zeros(total, np.int16)
    oidx = np.zeros(total, np.int16)
    rel = np.full(total, 128, np.int16)
    gidx[pos] = (ob - hb * HALF).astype(np.int16)
    oidx[pos] = (kb - core * rpc).astype(np.int16)
    rel[pos] = (kb & 127).astype(np.int16)

    per_core = []
    per = ntiles * 128
    for c in range(n_cores):
        sl = slice(c * per, (c + 1) * per)
        g = _wrap16(gidx[sl])
        o = _wrap16(oidx[sl])
        r = np.ascontiguousarray(
            rel[sl].reshape(ntiles, 128).T.astype(np.float32)
        )
        per_core.append((g, o, r))
    return T_lo, T_hi, chunks, per_core


def _splits(t0, nt, maxt):
    out = []
    while nt > 0:
        n = min(nt, maxt)
        out.append((t0, n))
        t0 += n
        nt -= n
    return out


def _build_bass_program(npad, rpc, nblk, t1_lo, t1_hi, chunks1, t2_lo, t2_hi,
                        chunks2, n_cores, enable_asserts=False):
    import concourse.bacc as bacc
    import concourse.mybir as mybir
    import concourse.tile as tile

    dt = mybir.dt
    Alu = mybir.AluOpType
    Act = mybir.ActivationFunctionType
    T1 = t1_lo + t1_hi
    T2 = t2_lo + t2_hi
    nt1 = nblk * T1
    nt2 = nblk * T2
    KC = IN_DIM // 128
    WCOL = F + 2 * HEADS  # 264
    H2 = 2 * HEADS
    bf16 = dt.bfloat16

    nc = bacc.Bacc(
        "TRN2",
        target_bir_lowering=False,
        debug=False,
        enable_asserts=enable_asserts,
        num_devices=n_cores,
        num_swdge_queues=4,
    )

    xT = nc.dram_tensor("xT", [IN_DIM, rpc], bf16, kind="ExternalInput")
    W_in = nc.dram_tensor("W", [IN_DIM, F], bf16, kind="ExternalInput")
    WT_in = nc.dram_tensor("WT", [F, IN_DIM], bf16, kind="ExternalInput")
    a_in = nc.dram_tensor("a", [HEADS, 2 * OUT_DIM], bf16, kind="ExternalInput")
    bias_in = nc.dram_tensor("bias", [1, F], dt.float32, kind="ExternalInput")
    p1_gidx = nc.dram_tensor("p1_gidx", [128, nt1 * 8], dt.int16, kind="ExternalInput")
    p1_oidx = nc.dram_tensor("p1_oidx", [128, nt1 * 8], dt.int16, kind="ExternalInput")
    p1_rel = nc.dram_tensor("p1_rel", [128, nt1], dt.float32, kind="ExternalInput")
    p2_gidx = nc.dram_tensor("p2_gidx", [128, nt2 * 8], dt.int16, kind="ExternalInput")
    p2_oidx = nc.dram_tensor("p2_oidx", [128, nt2 * 8], dt.int16, kind="ExternalInput")
    p2_rel = nc.dram_tensor("p2_rel", [128, nt2], dt.float32, kind="ExternalInput")
    out = nc.dram_tensor("out", [rpc, F], dt.float32, kind="ExternalOutput")

    with tile.TileContext(nc) as tc:
        with (
            tc.tile_pool(name="const", bufs=1) as cpool,
            tc.tile_pool(name="dram", bufs=1, space="DRAM") as dram,
        ):
            H_own = dram.tile([rpc, HROW], bf16)
            H_full = dram.tile([npad, HROW], bf16, addr_space="Shared")
            S_ownE = dram.tile([rpc, SROW], dt.float32)
            S_own8 = dram.tile([rpc, 8], dt.float32)
            S_all8 = dram.tile([npad, 8], dt.float32, addr_space="Shared")
            SR_full = dram.tile([npad, SROW], dt.float32)
            R_own = dram.tile([rpc, HEADS], dt.float32)
            R_all = dram.tile([npad, HEADS], dt.float32, addr_space="Shared")

            # ---------------- constants ----------------
            iota_i = cpool.tile([128, 128], dt.int32)
            nc.gpsimd.iota(iota_i[:], pattern=[[1, 128]], channel_multiplier=0)
            iota_f = cpool.tile([128, 128], dt.float32)
            nc.vector.tensor_copy(iota_f[:], iota_i[:])
            iota_bf = cpool.tile([128, 128], bf16)
            nc.vector.tensor_copy(iota_bf[:], iota_i[:])
            pidx_i = cpool.tile([128, 1], dt.int32)
            nc.gpsimd.iota(pidx_i[:], pattern=[[0, 1]], channel_multiplier=1)
            pidx_f = cpool.tile([128, 1], dt.float32)
            nc.vector.tensor_copy(pidx_f[:], pidx_i[:])
            ident = cpool.tile([128, 128], bf16)
            nc.vector.tensor_scalar(ident[:], iota_f[:], pidx_f[:], None, op0=Alu.is_equal)

            bias_bc = cpool.tile([128, F], dt.float32)
            nc.sync.dma_start(bias_bc[:1, :], bias_in[:, :])
            nc.gpsimd.partition_broadcast(bias_bc[:], bias_bc[:1, :])

            A0 = cpool.tile([128, H2], bf16)
            A1 = cpool.tile([128, H2], bf16)
            nc.vector.memset(A0[:], 0.0)
            nc.vector.memset(A1[:], 0.0)
            for h in range(HEADS):
                dstA = A0 if h < 2 else A1
                p0c = (h % 2) * OUT_DIM
                nc.sync.dma_start(dstA[p0c:p0c + OUT_DIM, h:h + 1], a_in[h:h + 1, 0:OUT_DIM])
                nc.sync.dma_start(
                    dstA[p0c:p0c + OUT_DIM, HEADS + h:HEADS + h + 1],
                    a_in[h:h + 1, OUT_DIM:2 * OUT_DIM],
                )

            W_sb = cpool.tile([128, KC * WCOL], bf16)
            for kc in range(KC):
                nc.sync.dma_start(
                    W_sb[:, kc * WCOL:kc * WCOL + F], W_in[kc * 128:(kc + 1) * 128, :]
                )
            WT0 = cpool.tile([128, IN_DIM], bf16)
            WT1 = cpool.tile([128, IN_DIM], bf16)
            nc.sync.dma_start(WT0[:], WT_in[0:128, :])
            nc.sync.dma_start(WT1[:], WT_in[128:256, :])
            with tc.tile_pool(name="psum_pre", bufs=2, space="PSUM") as pp:
                for kc in range(KC):
                    pwa = pp.tile([128, H2], dt.float32, tag="wa")
                    for fc in range(2):
                        wt = WT0 if fc == 0 else WT1
                        A = A0 if fc == 0 else A1
                        nc.tensor.matmul(
                            pwa[:], wt[:, kc * 128:(kc + 1) * 128], A[:],
                            start=(fc == 0), stop=(fc == 1),
                        )
                    nc.vector.tensor_copy(W_sb[:, kc * WCOL + F:(kc + 1) * WCOL], pwa[:])

            rec_all = cpool.tile([128, nblk * HEADS], dt.float32)

            # ---------------- phase 0: projection ----------------
            with (
                tc.tile_pool(name="p0x", bufs=1) as p0x,
                tc.tile_pool(name="p0", bufs=3) as p0pool,
                tc.tile_pool(name="p0ps", bufs=2, space="PSUM") as p0ps,
            ):
                xres = p0x.tile([128, KC * rpc], bf16)
                for kc in range(KC):
                    nc.sync.dma_start(
                        xres[:, kc * rpc:(kc + 1) * rpc],
                        xT[kc * 128:(kc + 1) * 128, :],
                    )
                for r in range(nblk):
                    ps = p0ps.tile([128, WCOL], dt.float32, tag="hps")
                    for kc in range(KC):
                        nc.tensor.matmul(
                            ps[:], xres[:, kc * rpc + r * 128:kc * rpc + (r + 1) * 128],
                            W_sb[:, kc * WCOL:(kc + 1) * WCOL],
                            start=(kc == 0), stop=(kc == KC - 1),
                        )
                    gsb = p0pool.tile([128, HROW], bf16, tag="gsb")
                    nc.vector.tensor_copy(gsb[:], ps[:, 0:F])  # h -> bf16
                    nc.sync.dma_start(H_own[r * 128:(r + 1) * 128, :], gsb[:])
                    ssb = p0pool.tile([128, 8], dt.float32, tag="ssb")
                    nc.vector.tensor_copy(ssb[:], ps[:, F:WCOL])
                    nc.sync.dma_start(S_ownE[r * 128:(r + 1) * 128, 0:8], ssb[:])
                    nc.sync.dma_start(S_own8[r * 128:(r + 1) * 128, :], ssb[:])

            # ---------------- collectives after phase 0 ----------------
            nc.gpsimd.collective_compute(
                "AllGather", Alu.bypass,
                ins=[S_own8[:].opt()], outs=[S_all8[:].opt()],
                replica_groups=[list(range(n_cores))],
            )
            nc.gpsimd.collective_compute(
                "AllGather", Alu.bypass,
                ins=[H_own[:].opt()], outs=[H_full[:].opt()],
                replica_groups=[list(range(n_cores))],
            )
            # expand compact s table into 256B-stride gather table
            nc.sync.dma_start(SR_full[:, 0:8], S_all8[:, :])

            # ---------------- phase 1: softmax denominators ----------------
            with (
                tc.tile_pool(name="p1i", bufs=2) as p1i,
                tc.tile_pool(name="p1", bufs=2) as p1pool,
                tc.tile_pool(name="p1eq", bufs=2) as p1eq,
                tc.tile_pool(name="p1v", bufs=3) as p1v,
                tc.tile_pool(name="p1ps", bufs=2, space="PSUM") as p1ps,
            ):
                qn = 0
                for ci, (c0, cb) in enumerate(chunks1):
                    tg0 = 0
                    for cc0, ccb in chunks1[:ci]:
                        tg0 += ccb * T1
                    ntl = cb * t1_lo
                    nth = cb * t1_hi
                    ntc = cb * T1
                    gix = p1i.tile([128, CB1 * T1 * 8], dt.int16, tag="gix")
                    nc.sync.dma_start(
                        gix[:, 0:ntc * 8], p1_gidx[:, tg0 * 8:(tg0 + ntc) * 8]
                    )
                    oix = p1i.tile([128, CB1 * T1 * 8], dt.int16, tag="oix")
                    nc.sync.dma_start(
                        oix[:, 0:ntc * 8], p1_oidx[:, tg0 * 8:(tg0 + ntc) * 8]
                    )
                    rels = p1i.tile([128, CB1 * T1], dt.float32, tag="rel")
                    nc.sync.dma_start(rels[:, 0:ntc], p1_rel[:, tg0:tg0 + ntc])

                    g1 = p1pool.tile([128, CB1 * T1, SROW], dt.float32, tag="g1")
                    own = p1pool.tile([128, CB1 * T1, SROW], dt.float32, tag="own")
                    for t0, n in _splits(0, ntl, MAXT1):
                        nc.gpsimd.dma_gather(
                            g1[:, t0:t0 + n, :], SR_full[0:HALF, :],
                            gix[:, t0 * 8:(t0 + n) * 8], n * 128, n * 128, SROW,
                            single_packet=False, queue_num=qn % 4,
                        )
                        qn += 1
                    for t0, n in _splits(ntl, nth, MAXT1):
                        nc.gpsimd.dma_gather(
                            g1[:, t0:t0 + n, :], SR_full[HALF:npad, :],
                            gix[:, t0 * 8:(t0 + n) * 8], n * 128, n * 128, SROW,
                            single_packet=False, queue_num=qn % 4,
                        )
                        qn += 1
                    for t0, n in _splits(0, ntc, MAXT1):
                        nc.gpsimd.dma_gather(
                            own[:, t0:t0 + n, :], S_ownE[:, :],
                            oix[:, t0 * 8:(t0 + n) * 8], n * 128, n * 128, SROW,
                            single_packet=False, queue_num=qn % 4,
                        )
                        qn += 1

                    eq = p1eq.tile([128, CB1 * T1, 128], bf16, tag="eq")
                    for ti in range(ntc):
                        nc.vector.tensor_scalar(
                            eq[:, ti, :], iota_f[:], rels[:, ti:ti + 1], None,
                            op0=Alu.is_equal,
                        )

                    for j in range(cb):
                        lo0 = j * t1_lo
                        hi0 = ntl + j * t1_hi
                        z = p1v.tile([128, T1, HEADS], dt.float32, tag="z")
                        nc.vector.tensor_tensor(
                            z[:, 0:t1_lo, :], own[:, lo0:lo0 + t1_lo, 0:HEADS],
                            g1[:, lo0:lo0 + t1_lo, HEADS:H2], op=Alu.add,
                        )
                        if t1_hi:
                            nc.vector.tensor_tensor(
                                z[:, t1_lo:T1, :], own[:, hi0:hi0 + t1_hi, 0:HEADS],
                                g1[:, hi0:hi0 + t1_hi, HEADS:H2], op=Alu.add,
                            )
                        zf = z[:].rearrange("p t h -> p (t h)")
                        nc.vector.scalar_tensor_tensor(
                            zf, zf, 0.2, zf, op0=Alu.mult, op1=Alu.max
                        )
                        v = p1v.tile([128, T1 * HEADS], bf16, tag="v")
                        nc.scalar.activation(v[:], zf, Act.Exp)

                        ps1 = p1ps.tile([128, HEADS], dt.float32, tag="ps1")
                        for t in range(T1):
                            ti = lo0 + t if t < t1_lo else hi0 + (t - t1_lo)
                            nc.tensor.matmul(
                                ps1[:], eq[:, ti, :],
                                v[:, t * HEADS:(t + 1) * HEADS],
                                start=(t == 0), stop=(t == T1 - 1),
                            )
                        nc.vector.tensor_scalar_add(
                            rec_all[:, (c0 + j) * HEADS:(c0 + j + 1) * HEADS],
                            ps1[:], 1e-10,
                        )
                recd = cpool.tile([128, nblk * HEADS], dt.float32)
                nc.vector.reciprocal(recd[:], rec_all[:])
                nc.sync.dma_start(
                    R_own[:].rearrange("(b p) h -> p b h", p=128),
                    recd[:].rearrange("p (b h) -> p b h", h=HEADS),
                )

            nc.gpsimd.collective_compute(
                "AllGather", Alu.bypass,
                ins=[R_own[:].opt()], outs=[R_all[:].opt()],
                replica_groups=[list(range(n_cores))],
            )
            # write reciprocals into the recip slot of every SR_full row
            nc.sync.dma_start(SR_full[:, 8:12], R_all[:, :])

            # ---------------- phase 2: aggregate messages ----------------
            with (
                tc.tile_pool(name="p2i", bufs=2) as p2i,
                tc.tile_pool(name="p2", bufs=2) as p2pool,
                tc.tile_pool(name="p2eq", bufs=2) as p2eq,
                tc.tile_pool(name="p2v", bufs=3) as p2v,
                tc.tile_pool(name="p2m", bufs=4) as p2m,
                tc.tile_pool(name="p2ps", bufs=2, space="PSUM") as p2ps,
            ):
                qn = 0
                eqn = 0
                for ci, (c0, cb) in enumerate(chunks2):
                    tg0 = 0
                    for cc0, ccb in chunks2[:ci]:
                        tg0 += ccb * T2
                    ntl = cb * t2_lo
                    nth = cb * t2_hi
                    ntc = cb * T2
                    gix = p2i.tile([128, CB2 * T2 * 8], dt.int16, tag="gix")
                    nc.sync.dma_start(
                        gix[:, 0:ntc * 8], p2_gidx[:, tg0 * 8:(tg0 + ntc) * 8]
                    )
                    oix = p2i.tile([128, CB2 * T2 * 8], dt.int16, tag="oix")
                    nc.sync.dma_start(
                        oix[:, 0:ntc * 8], p2_oidx[:, tg0 * 8:(tg0 + ntc) * 8]
                    )
                    rels = p2i.tile([128, CB2 * T2], dt.float32, tag="rel")
                    nc.sync.dma_start(rels[:, 0:ntc], p2_rel[:, tg0:tg0 + ntc])

                    g = p2pool.tile([128, CB2 * T2, HROW], bf16, tag="g")
                    rs = p2pool.tile([128, CB2 * T2, SROW], dt.float32, tag="rs")
                    own = p2pool.tile([128, CB2 * T2, SROW], dt.float32, tag="own")
                    for t0, n in _splits(0, ntl, MAXT2):
                        nc.gpsimd.dma_gather(
                            g[:, t0:t0 + n, :], H_full[0:HALF, :],
                            gix[:, t0 * 8:(t0 + n) * 8], n * 128, n * 128, HROW,
                            single_packet=False, queue_num=qn % 4,
                        )
                        qn += 1
                        nc.gpsimd.dma_gather(
                            rs[:, t0:t0 + n, :], SR_full[0:HALF, :],
                            gix[:, t0 * 8:(t0 + n) * 8], n * 128, n * 128, SROW,
                            single_packet=False, queue_num=qn % 4,
                        )
                        qn += 1
                    for t0, n in _splits(ntl, nth, MAXT2):
                        nc.gpsimd.dma_gather(
                            g[:, t0:t0 + n, :], H_full[HALF:npad, :],
                            gix[:, t0 * 8:(t0 + n) * 8], n * 128, n * 128, HROW,
                            single_packet=False, queue_num=qn % 4,
                        )
                        qn += 1
                        nc.gpsimd.dma_gather(
                            rs[:, t0:t0 + n, :], SR_full[HALF:npad, :],
                            gix[:, t0 * 8:(t0 + n) * 8], n * 128, n * 128, SROW,
                            single_packet=False, queue_num=qn % 4,
                        )
                        qn += 1
                    for t0, n in _splits(0, ntc, MAXT1):
                        nc.gpsimd.dma_gather(
                            own[:, t0:t0 + n, :], S_ownE[:, :],
                            oix[:, t0 * 8:(t0 + n) * 8], n * 128, n * 128, SROW,
                            single_packet=False, queue_num=qn % 4,
                        )
                        qn += 1

                    eq = p2eq.tile([128, CB2 * T2, 128], bf16, tag="eq")
                    for ti in range(ntc):
                        eng = nc.vector if (eqn % 2 == 0) else nc.gpsimd
                        eng.tensor_scalar(
                            eq[:, ti, :], iota_f[:], rels[:, ti:ti + 1], None,
                            op0=Alu.is_equal,
                        )
                        eqn += 1

                    for j in range(cb):
                        lo0 = j * t2_lo
                        hi0 = ntl + j * t2_hi
                        al = p2v.tile([128, T2, HEADS], dt.float32, tag="al")
                        nc.vector.tensor_tensor(
                            al[:, 0:t2_lo, :],
                            rs[:, lo0:lo0 + t2_lo, 0:HEADS],
                            own[:, lo0:lo0 + t2_lo, HEADS:H2], op=Alu.add,
                        )
                        if t2_hi:
                            nc.vector.tensor_tensor(
                                al[:, t2_lo:T2, :],
                                rs[:, hi0:hi0 + t2_hi, 0:HEADS],
                                own[:, hi0:hi0 + t2_hi, HEADS:H2], op=Alu.add,
                            )
                        alf = al[:].rearrange("p t h -> p (t h)")
                        nc.vector.scalar_tensor_tensor(
                            alf, alf, 0.2, alf, op0=Alu.mult, op1=Alu.max
                        )
                        nc.scalar.activation(alf, alf, Act.Exp)
                        nc.vector.tensor_tensor(
                            al[:, 0:t2_lo, :], al[:, 0:t2_lo, :],
                            rs[:, lo0:lo0 + t2_lo, 8:12], op=Alu.mult,
                        )
                        if t2_hi:
                            nc.vector.tensor_tensor(
                                al[:, t2_lo:T2, :], al[:, t2_lo:T2, :],
                                rs[:, hi0:hi0 + t2_hi, 8:12], op=Alu.mult,
                            )
                        alb = p2v.tile([128, T2 * HEADS], bf16, tag="alb")
                        nc.vector.tensor_copy(alb[:], alf)

                        ps2 = p2ps.tile([128, F], dt.float32, tag="ps2")
                        for t in range(T2):
                            ti = lo0 + t if t < t2_lo else hi0 + (t - t2_lo)
                            alpha_b = alb[:, t * HEADS:(t + 1) * HEADS].unsqueeze(
                                2
                            ).broadcast_to([128, HEADS, OUT_DIM])
                            msg = p2m.tile([128, F], bf16, tag="msg")
                            eng = nc.vector if (t % 2 == 0) else nc.gpsimd
                            eng.tensor_tensor(
                                msg[:].rearrange("p (h d) -> p h d", h=HEADS),
                                g[:, ti, :].rearrange("p (h d) -> p h d", h=HEADS),
                                alpha_b,
                                op=Alu.mult,
                            )
                            nc.tensor.matmul(
                                ps2[:], eq[:, ti, :], msg[:],
                                start=(t == 0), stop=(t == T2 - 1),
                            )
                        osb = p2m.tile([128, F], dt.float32, tag="osb")
                        nc.vector.tensor_tensor(
                            osb[:], ps2[:], bias_bc[:], op=Alu.add
                        )
                        nc.sync.dma_start(
                            out[(c0 + j) * 128:(c0 + j + 1) * 128, :], osb[:]
                        )

    nc.compile()
    return nc


def _gat_forward(x, edges, W, a, bias, n_nodes, n_cores, run_opts=None):
    import ml_dtypes

    npad = _ceil_div(n_nodes, n_cores * 128) * n_cores * 128
    rpc = npad // n_cores
    nblk = rpc // 128

    src = edges[:, 0].astype(np.int64)
    dst = edges[:, 1].astype(np.int64)
    t1_lo, t1_hi, chunks1, meta1 = _build_phase_meta(src, dst, n_cores, nblk, CB1)
    t2_lo, t2_hi, chunks2, meta2 = _build_phase_meta(dst, src, n_cores, nblk, CB2)

    nc = _build_bass_program(
        npad, rpc, nblk, t1_lo, t1_hi, chunks1, t2_lo, t2_hi, chunks2, n_cores
    )

    x_pad = np.zeros((npad, IN_DIM), np.float32)
    x_pad[:n_nodes] = x
    xT = np.ascontiguousarray(x_pad.T).astype(ml_dtypes.bfloat16)
    W_b = np.ascontiguousarray(W.astype(np.float32)).astype(ml_dtypes.bfloat16)
    WT_b = np.ascontiguousarray(W_b.T)

    in_maps = []
    for c in range(n_cores):
        g1, o1, r1 = meta1[c]
        g2, o2, r2 = meta2[c]
        in_maps.append({
            "xT": np.ascontiguousarray(xT[:, c * rpc:(c + 1) * rpc]),
            "W": W_b,
            "WT": WT_b,
            "a": np.ascontiguousarray(a.astype(np.float32)).astype(ml_dtypes.bfloat16),
            "bias": np.ascontiguousarray(bias.astype(np.float32).reshape(1, F)),
            "p1_gidx": g1, "p1_oidx": o1, "p1_rel": r1,
            "p2_gidx": g2, "p2_oidx": o2, "p2_rel": r2,
        })

    from concourse.bass_utils import run_bass_kernel_spmd

    res = run_bass_kernel_spmd(
        nc, in_maps, core_ids=list(range(n_cores)), **(run_opts or {})
    )
    out = np.concatenate([r["out"] for r in res.results], axis=0)
    return out[:n_nodes], res


def kernel(x, edges, W, a, bias):
    x = np.asarray(x, np.float32)
    edges = np.asarray(edges)
    W = np.asarray(W, np.float32)
    a = np.asarray(a, np.float32)
    bias = np.asarray(bias, np.float32)
    out, _ = _gat_forward(x, edges, W, a, bias, N_NODES, N_CORES)
    return out


# revision 14
# speedup vs baseline: 2.4364x; 2.4364x over previous
"""GATConv (multi-head graph attention) on 8 Trainium2 NeuronCores.

kernel(**inputs) takes the FULL numpy inputs and returns the FULL
[50000, 256] float32 output.  All floating-point math runs on-device in a
Bass/Tile kernel; the host only does index bookkeeping (node remapping,
edge sorting, gather-index tables, padding) and dtype conversion.

Distribution: nodes are degree-sorted, then block-interleaved across the
8 cores (so the 8 cores' r-th blocks have near-identical degree
profiles and share compile-time tile counts).  Per core:
 - phase 0 projects its node slice (x @ [W | W@A], bf16) -> h + attention
   dot-products s.
 - phase 1 (degree-major): lanes = source nodes, free slots = their
   out-edges; one dma_gather stream fetches s_dst rows; s_src is a
   per-lane broadcast of the resident s table; denominators come from a
   free-dim reduce.  No selector matmuls, no own-node gathers.
 - phase 2 (scatter-form): edges grouped by dst block; one combined
   768B-row gather (h | s_src | recip) + an own-node s_dst gather; the
   one-hot scatter matrices are built on-chip from 2-byte rel indices
   and applied on the PE.
The big h AllGather overlaps phase 1; reciprocals arrive afterwards via
a tiny compact AllGather plus strided DRAM->DRAM writes into the local
combined-row table.
"""

import sys

sys.path.insert(0, "/opt/trn_rl_repo")

import numpy as np

N_NODES = 50000
N_EDGES = 800000
IN_DIM = 512
HEADS = 4
OUT_DIM = 64
F = HEADS * OUT_DIM  # 256
N_CORES = 8
HALF = 32768  # int16 gather index range split
GROW = 384  # G2 table bf16 elems/row: 256 h | 8 (s_src f32) | 8 (recip f32) | pad
SROW = 64  # SR table f32 elems/row: 4 s_src | 4 s_dst | pad -> 256B

CB2 = 3  # blocks per chunk, phase 2
MAXT = 24  # max tiles per dma_gather call, 256B rows
MAXTG = 16  # max tiles per call for 768B G2 rows
K1MAX = 48  # max slots per lane in phase 1 (tile allocation bound)


def _ceil_div(a, b):
    return (a + b - 1) // b


def _wrap16(arr_i16):
    """dma_gather idx layout: position i -> [i % 16, i // 16], x8 core groups."""
    n = arr_i16.shape[0]
    assert n % 16 == 0
    w = arr_i16.reshape(n // 16, 16).T
    return np.ascontiguousarray(np.tile(w, (8, 1)))


def _chunk_plan(nblk, cb_max):
    chunks = []
    c0 = 0
    while c0 < nblk:
        cb = min(cb_max, nblk - c0)
        chunks.append((c0, cb))
        c0 += cb
    return chunks


def _splits(t0, nt, maxt):
    out = []
    while nt > 0:
        n = min(nt, maxt)
        out.append((t0, n))
        t0 += n
        nt -= n
    return out


def _build_p1_meta(src, dst, n_cores, nblk):
    """Degree-major phase-1 layout.

    Per (core, block r): lanes = the block's 128 nodes; slot stream =
    [K_lo[r] low-half slots | K_hi[r] high-half slots] x 128 lanes,
    position (t*128+p) = slot t of lane p.  idx = dst (half-local),
    pad slots idx 0 (gather row 0; contribution masked by vmask).

    Returns (K_lo[r], K_hi[r], gidx[c] [128, ntot*8] i16,
             vmask[c] [128, ntot] f32  (1 real / 0 pad))
    """
    rpc = nblk * 128
    hi = (dst >= HALF).astype(np.int64)
    order = np.lexsort((dst, hi, src))
    ss, ds, hs = src[order], dst[order], hi[order]
    nh = ss * 2 + hs
    counts = np.bincount(nh, minlength=n_cores * rpc * 2)
    starts = np.zeros(len(counts), np.int64)
    np.cumsum(counts[:-1], out=starts[1:])
    rank = np.arange(len(ss), dtype=np.int64) - starts[nh]

    cnt2 = counts.reshape(n_cores, nblk, 128, 2)
    K_lo = cnt2[:, :, :, 0].max(axis=(0, 2))  # [nblk] max over cores & lanes
    K_hi = cnt2[:, :, :, 1].max(axis=(0, 2))
    K_lo = np.maximum(K_lo, 1)
    K = K_lo + K_hi
    base = np.zeros(nblk + 1, np.int64)
    np.cumsum(K, out=base[1:])
    ntot = int(base[-1])  # slot-tiles per core

    core = ss // rpc
    blk = (ss % rpc) >> 7
    lane = ss & 127
    slot = np.where(hs == 0, rank, K_lo[blk] + rank)
    pos = (core * ntot + base[blk] + slot) * 128 + lane

    total = n_cores * ntot * 128
    gidx = np.zeros(total, np.int16)
    gidx[pos] = (ds - hs * HALF).astype(np.int16)
    vm = np.zeros(total, np.float32)
    vm[pos] = 1.0

    per = ntot * 128
    gidx_c, vm_c = [], []
    for c in range(n_cores):
        sl = slice(c * per, (c + 1) * per)
        gidx_c.append(_wrap16(gidx[sl]))
        vm_c.append(np.ascontiguousarray(vm[sl].reshape(ntot, 128).T))
    return K_lo.astype(int), K_hi.astype(int), gidx_c, vm_c


def _build_p2_meta(key, other, n_cores, nblk, cb_max):
    """Phase-2 (scatter-form) meta: chunked tile layout.

    Edges grouped by key block; chunk of cb blocks lays tiles out as
    [cb*T_lo low tiles | cb*T_hi high tiles]; block j's low tiles start
    at j*T_lo, its high tiles at cb*T_lo + j*T_hi.
    """
    rpc = nblk * 128
    gblk = (key >> 7).astype(np.int64)
    hi = (other >= HALF).astype(np.int64)
    order = np.lexsort((other, hi, gblk))
    kb, ob, hb, gb = key[order], other[order], hi[order], gblk[order]

    nrun = n_cores * nblk * 2
    run = gb * 2 + hb
    counts = np.bincount(run, minlength=nrun)
    T_lo = max(1, int(np.max(_ceil_div(counts[0::2], 128))))
    T_hi = int(np.max(_ceil_div(counts[1::2], 128)))
    T = T_lo + T_hi

    chunks = _chunk_plan(nblk, cb_max)
    ntiles = nblk * T
    tb = np.zeros(len(chunks) + 1, np.int64)
    for i, (_, cb_) in enumerate(chunks):
        tb[i + 1] = tb[i] + cb_ * T

    starts = np.zeros(nrun, np.int64)
    np.cumsum(counts[:-1], out=starts[1:])
    rank = np.arange(len(kb), dtype=np.int64) - starts[run]
    blocal = gb % nblk
    core = gb // nblk
    chunk_id = blocal // cb_max
    c0_arr = chunk_id * cb_max
    cb_arr = np.minimum(cb_max, nblk - c0_arr)
    tic = np.where(
        hb == 0,
        (blocal - c0_arr) * T_lo + rank // 128,
        cb_arr * T_lo + (blocal - c0_arr) * T_hi + rank // 128,
    )
    tile = tb[chunk_id] + tic
    pos = (core * ntiles + tile) * 128 + rank % 128

    total = n_cores * ntiles * 128
    gidx = np.zeros(total, np.int16)
    oidx = np.zeros(total, np.int16)
    rel = np.full(total, 128, np.int16)
    gidx[pos] = (ob - hb * HALF).astype(np.int16)
    oidx[pos] = (kb - core * rpc).astype(np.int16)
    rel[pos] = (kb & 127).astype(np.int16)

    per_core = []
    per = ntiles * 128
    for c in range(n_cores):
        sl = slice(c * per, (c + 1) * per)
        per_core.append((
            _wrap16(gidx[sl]),
            _wrap16(oidx[sl]),
            np.ascontiguousarray(rel[sl].reshape(ntiles, 128).T.astype(np.float32)),
        ))
    return T_lo, T_hi, chunks, per_core


def _build_bass_program(npad, rpc, nblk, k1_lo, k1_hi, t2_lo, t2_hi, chunks2,
                        n_cores, enable_asserts=False):
    import concourse.bacc as bacc
    import concourse.mybir as mybir
    import concourse.tile as tile

    dt = mybir.dt
    Alu = mybir.AluOpType
    Act = mybir.ActivationFunctionType
    T2 = t2_lo + t2_hi
    nt2 = nblk * T2
    K1 = [int(k1_lo[r] + k1_hi[r]) for r in range(nblk)]
    base1 = [0]
    for r in range(nblk):
        base1.append(base1[-1] + K1[r])
    nt1 = base1[-1]
    KC = IN_DIM // 128
    WCOL = F + 2 * HEADS  # 264
    H2 = 2 * HEADS
    bf16 = dt.bfloat16

    nc = bacc.Bacc(
        "TRN2",
        target_bir_lowering=False,
        debug=False,
        enable_asserts=enable_asserts,
        num_devices=n_cores,
        num_swdge_queues=4,
    )

    xT = nc.dram_tensor("xT", [IN_DIM, rpc], bf16, kind="ExternalInput")
    W_in = nc.dram_tensor("W", [IN_DIM, F], bf16, kind="ExternalInput")
    WT_in = nc.dram_tensor("WT", [F, IN_DIM], bf16, kind="ExternalInput")
    a_in = nc.dram_tensor("a", [HEADS, 2 * OUT_DIM], bf16, kind="ExternalInput")
    bias_in = nc.dram_tensor("bias", [1, F], dt.float32, kind="ExternalInput")
    p1_gidx = nc.dram_tensor("p1_gidx", [128, nt1 * 8], dt.int16, kind="ExternalInput")
    p1_vm = nc.dram_tensor("p1_vm", [128, nt1], dt.float32, kind="ExternalInput")
    p2_gidx = nc.dram_tensor("p2_gidx", [128, nt2 * 8], dt.int16, kind="ExternalInput")
    p2_oidx = nc.dram_tensor("p2_oidx", [128, nt2 * 8], dt.int16, kind="ExternalInput")
    p2_rel = nc.dram_tensor("p2_rel", [128, nt2], dt.float32, kind="ExternalInput")
    out = nc.dram_tensor("out", [rpc, F], dt.float32, kind="ExternalOutput")

    with tile.TileContext(nc) as tc:
        with (
            tc.tile_pool(name="const", bufs=1) as cpool,
            tc.tile_pool(name="dram", bufs=1, space="DRAM") as dram,
        ):
            H_own = dram.tile([rpc, F], bf16)
            H_full = dram.tile([npad, F], bf16, addr_space="Shared")
            S_ownE = dram.tile([rpc, SROW], dt.float32)
            S_own8 = dram.tile([rpc, 8], dt.float32)
            S_all8 = dram.tile([npad, 8], dt.float32, addr_space="Shared")
            SR_full = dram.tile([npad, SROW], dt.float32)
            G2 = dram.tile([npad, GROW], bf16)
            R_own = dram.tile([rpc, HEADS], dt.float32)
            R_all = dram.tile([npad, HEADS], dt.float32, addr_space="Shared")

            # ---------------- constants ----------------
            iota_i = cpool.tile([128, 128], dt.int32)
            nc.gpsimd.iota(iota_i[:], pattern=[[1, 128]], channel_multiplier=0)
            iota_f = cpool.tile([128, 128], dt.float32)
            nc.vector.tensor_copy(iota_f[:], iota_i[:])

            bias_bc = cpool.tile([128, F], dt.float32)
            nc.sync.dma_start(bias_bc[:1, :], bias_in[:, :])
            nc.gpsimd.partition_broadcast(bias_bc[:], bias_bc[:1, :])

            A0 = cpool.tile([128, H2], bf16)
            A1 = cpool.tile([128, H2], bf16)
            nc.vector.memset(A0[:], 0.0)
            nc.vector.memset(A1[:], 0.0)
            for h in range(HEADS):
                dstA = A0 if h < 2 else A1
                p0c = (h % 2) * OUT_DIM
                nc.sync.dma_start(dstA[p0c:p0c + OUT_DIM, h:h + 1], a_in[h:h + 1, 0:OUT_DIM])
                nc.sync.dma_start(
                    dstA[p0c:p0c + OUT_DIM, HEADS + h:HEADS + h + 1],
                    a_in[h:h + 1, OUT_DIM:2 * OUT_DIM],
                )

            W_sb = cpool.tile([128, KC * WCOL], bf16)
            for kc in range(KC):
                nc.sync.dma_start(
                    W_sb[:, kc * WCOL:kc * WCOL + F], W_in[kc * 128:(kc + 1) * 128, :]
                )
            WT0 = cpool.tile([128, IN_DIM], bf16)
            WT1 = cpool.tile([128, IN_DIM], bf16)
            nc.sync.dma_start(WT0[:], WT_in[0:128, :])
            nc.sync.dma_start(WT1[:], WT_in[128:256, :])
            with tc.tile_pool(name="psum_pre", bufs=2, space="PSUM") as pp:
                for kc in range(KC):
                    pwa = pp.tile([128, H2], dt.float32, tag="wa")
                    for fc in range(2):
                        wt = WT0 if fc == 0 else WT1
                        A = A0 if fc == 0 else A1
                        nc.tensor.matmul(
                            pwa[:], wt[:, kc * 128:(kc + 1) * 128], A[:],
                            start=(fc == 0), stop=(fc == 1),
                        )
                    nc.vector.tensor_copy(W_sb[:, kc * WCOL + F:(kc + 1) * WCOL], pwa[:])

            # s_src of own nodes, resident (per-lane broadcast source in p1)
            s_res = cpool.tile([128, nblk * HEADS], dt.float32)
            rec_all = cpool.tile([128, nblk * HEADS], dt.float32)

            # ---------------- phase 0: projection ----------------
            with (
                tc.tile_pool(name="p0x", bufs=1) as p0x,
                tc.tile_pool(name="p0", bufs=3) as p0pool,
                tc.tile_pool(name="p0ps", bufs=2, space="PSUM") as p0ps,
            ):
                xres = p0x.tile([128, KC * rpc], bf16)
                for kc in range(KC):
                    nc.sync.dma_start(
                        xres[:, kc * rpc:(kc + 1) * rpc],
                        xT[kc * 128:(kc + 1) * 128, :],
                    )
                for r in range(nblk):
                    ps = p0ps.tile([128, WCOL], dt.float32, tag="hps")
                    for kc in range(KC):
                        nc.tensor.matmul(
                            ps[:], xres[:, kc * rpc + r * 128:kc * rpc + (r + 1) * 128],
                            W_sb[:, kc * WCOL:(kc + 1) * WCOL],
                            start=(kc == 0), stop=(kc == KC - 1),
                        )
                    gsb = p0pool.tile([128, F], bf16, tag="gsb")
                    nc.vector.tensor_copy(gsb[:], ps[:, 0:F])
                    nc.sync.dma_start(H_own[r * 128:(r + 1) * 128, :], gsb[:])
                    ssb = p0pool.tile([128, 8], dt.float32, tag="ssb")
                    nc.vector.tensor_copy(ssb[:], ps[:, F:WCOL])
                    nc.sync.dma_start(S_ownE[r * 128:(r + 1) * 128, 0:8], ssb[:])
                    nc.sync.dma_start(S_own8[r * 128:(r + 1) * 128, :], ssb[:])
                    nc.vector.tensor_copy(
                        s_res[:, r * HEADS:(r + 1) * HEADS], ps[:, F:F + HEADS]
                    )

            # ---------------- collectives after phase 0 ----------------
            nc.gpsimd.collective_compute(
                "AllGather", Alu.bypass,
                ins=[S_own8[:].opt()], outs=[S_all8[:].opt()],
                replica_groups=[list(range(n_cores))],
            )
            nc.gpsimd.collective_compute(
                "AllGather", Alu.bypass,
                ins=[H_own[:].opt()], outs=[H_full[:].opt()],
                replica_groups=[list(range(n_cores))],
            )
            # expand compact s into the 256B-stride gather table (for p1)
            nc.sync.dma_start(SR_full[:, 0:8], S_all8[:, :])
            # assemble the combined phase-2 row table G2 (h part + s_src part);
            # consumed only by phase 2, so these overlap phase 1
            nc.sync.dma_start(G2[:, 0:F], H_full[:, :])
            nc.sync.dma_start(G2[:, F:F + H2].bitcast(dt.float32), S_all8[:, 0:4])

            # ---------------- phase 1: denominators (degree-major) ----------
            with (
                tc.tile_pool(name="p1i", bufs=2) as p1i,
                tc.tile_pool(name="p1", bufs=2) as p1pool,
                tc.tile_pool(name="p1v", bufs=3) as p1v,
            ):
                qn = 0
                for r in range(nblk):
                    klo = int(k1_lo[r])
                    khi = int(k1_hi[r])
                    kk = klo + khi
                    b0 = base1[r]
                    gix = p1i.tile([128, K1MAX * 8], dt.int16, tag="gix")
                    nc.sync.dma_start(
                        gix[:, 0:kk * 8], p1_gidx[:, b0 * 8:(b0 + kk) * 8]
                    )
                    vms = p1i.tile([128, K1MAX], dt.float32, tag="vm")
                    nc.sync.dma_start(vms[:, 0:kk], p1_vm[:, b0:b0 + kk])

                    g1 = p1pool.tile([128, K1MAX, SROW], dt.float32, tag="g1")
                    for t0, n in _splits(0, klo, MAXT):
                        nc.gpsimd.dma_gather(
                            g1[:, t0:t0 + n, :], SR_full[0:HALF, :],
                            gix[:, t0 * 8:(t0 + n) * 8], n * 128, n * 128, SROW,
                            single_packet=False, queue_num=qn % 4,
                        )
                        qn += 1
                    for t0, n in _splits(klo, khi, MAXT):
                        nc.gpsimd.dma_gather(
                            g1[:, t0:t0 + n, :], SR_full[HALF:npad, :],
                            gix[:, t0 * 8:(t0 + n) * 8], n * 128, n * 128, SROW,
                            single_packet=False, queue_num=qn % 4,
                        )
                        qn += 1

                    # z[p, k, h] = s_res[p, r, h] + s_dst[p, k, h]
                    z = p1v.tile([128, K1MAX, HEADS], dt.float32, tag="z")
                    nc.vector.tensor_tensor(
                        z[:, 0:kk, :],
                        g1[:, 0:kk, HEADS:H2],
                        s_res[:, r * HEADS:(r + 1) * HEADS].unsqueeze(1)
                            .broadcast_to([128, kk, HEADS]),
                        op=Alu.add,
                    )
                    zf = z[:, 0:kk, :].rearrange("p t h -> p (t h)")
                    nc.vector.scalar_tensor_tensor(
                        zf, zf, 0.2, zf, op0=Alu.mult, op1=Alu.max
                    )
                    nc.scalar.activation(zf, zf, Act.Exp)
                    # mask pads, then reduce over slots
                    nc.vector.tensor_tensor(
                        z[:, 0:kk, :], z[:, 0:kk, :],
                        vms[:, 0:kk].unsqueeze(2).broadcast_to([128, kk, HEADS]),
                        op=Alu.mult,
                    )
                    nc.vector.tensor_reduce(
                        rec_all[:, r * HEADS:(r + 1) * HEADS],
                        z[:, 0:kk, :].rearrange("p t h -> p h t"),
                        axis=mybir.AxisListType.X, op=Alu.add,
                    )
                recd = cpool.tile([128, nblk * HEADS], dt.float32)
                nc.vector.tensor_scalar_add(rec_all[:], rec_all[:], 1e-10)
                nc.vector.reciprocal(recd[:], rec_all[:])
                nc.sync.dma_start(
                    R_own[:].rearrange("(b p) h -> p b h", p=128),
                    recd[:].rearrange("p (b h) -> p b h", h=HEADS),
                )

            nc.gpsimd.collective_compute(
                "AllGather", Alu.bypass,
                ins=[R_own[:].opt()], outs=[R_all[:].opt()],
                replica_groups=[list(range(n_cores))],
            )
            # write reciprocals into the recip slot of every G2 row
            nc.sync.dma_start(G2[:, F + H2:F + 2 * H2].bitcast(dt.float32), R_all[:, :])

            # ---------------- phase 2: aggregate messages ----------------
            with (
                tc.tile_pool(name="p2i", bufs=2) as p2i,
                tc.tile_pool(name="p2", bufs=2) as p2pool,
                tc.tile_pool(name="p2eq", bufs=2) as p2eq,
                tc.tile_pool(name="p2v", bufs=3) as p2v,
                tc.tile_pool(name="p2m", bufs=2) as p2m,
                tc.tile_pool(name="p2ps", bufs=2, space="PSUM") as p2ps,
            ):
                qn = 0
                for ci, (c0, cb) in enumerate(chunks2):
                    tg0 = 0
                    for cc0, ccb in chunks2[:ci]:
                        tg0 += ccb * T2
                    ntl = cb * t2_lo
                    nth = cb * t2_hi
                    ntc = cb * T2
                    gix = p2i.tile([128, CB2 * T2 * 8], dt.int16, tag="gix")
                    nc.sync.dma_start(
                        gix[:, 0:ntc * 8], p2_gidx[:, tg0 * 8:(tg0 + ntc) * 8]
                    )
                    oix = p2i.tile([128, CB2 * T2 * 8], dt.int16, tag="oix")
                    nc.sync.dma_start(
                        oix[:, 0:ntc * 8], p2_oidx[:, tg0 * 8:(tg0 + ntc) * 8]
                    )
                    rels = p2i.tile([128, CB2 * T2], dt.float32, tag="rel")
                    nc.sync.dma_start(rels[:, 0:ntc], p2_rel[:, tg0:tg0 + ntc])

                    g = p2pool.tile([128, CB2 * T2, GROW], bf16, tag="g")
                    own = p2pool.tile([128, CB2 * T2, SROW], dt.float32, tag="own")
                    for t0, n in _splits(0, ntl, MAXTG):
                        nc.gpsimd.dma_gather(
                            g[:, t0:t0 + n, :], G2[0:HALF, :],
                            gix[:, t0 * 8:(t0 + n) * 8], n * 128, n * 128, GROW,
                            single_packet=False, queue_num=qn % 4,
                        )
                        qn += 1
                    for t0, n in _splits(ntl, nth, MAXTG):
                        nc.gpsimd.dma_gather(
                            g[:, t0:t0 + n, :], G2[HALF:npad, :],
                            gix[:, t0 * 8:(t0 + n) * 8], n * 128, n * 128, GROW,
                            single_packet=False, queue_num=qn % 4,
                        )
                        qn += 1
                    for t0, n in _splits(0, ntc, MAXT):
                        nc.gpsimd.dma_gather(
                            own[:, t0:t0 + n, :], S_ownE[:, :],
                            oix[:, t0 * 8:(t0 + n) * 8], n * 128, n * 128, SROW,
                            single_packet=False, queue_num=qn % 4,
                        )
                        qn += 1

                    # build all one-hot tiles of the chunk in one DVE op
                    eq = p2eq.tile([128, CB2 * T2, 128], bf16, tag="eq")
                    nc.vector.tensor_tensor(
                        eq[:, 0:ntc, :],
                        iota_f[:].unsqueeze(1).broadcast_to([128, ntc, 128]),
                        rels[:, 0:ntc].unsqueeze(2).broadcast_to([128, ntc, 128]),
                        op=Alu.is_equal,
                    )

                    # alpha chain, whole chunk at once
                    al = p2v.tile([128, CB2 * T2, HEADS], dt.float32, tag="al")
                    nc.vector.tensor_tensor(
                        al[:, 0:ntc, :],
                        g[:, 0:ntc, F:F + H2].bitcast(dt.float32),
                        own[:, 0:ntc, HEADS:H2], op=Alu.add,
                    )
                    alf = al[:, 0:ntc, :].rearrange("p t h -> p (t h)")
                    nc.vector.scalar_tensor_tensor(
                        alf, alf, 0.2, alf, op0=Alu.mult, op1=Alu.max
                    )
                    nc.scalar.activation(alf, alf, Act.Exp)
                    nc.vector.tensor_tensor(
                        al[:, 0:ntc, :], al[:, 0:ntc, :],
                        g[:, 0:ntc, F + H2:F + 2 * H2].bitcast(dt.float32),
                        op=Alu.mult,
                    )
                    alb = p2v.tile([128, CB2 * T2 * HEADS], bf16, tag="alb")
                    nc.vector.tensor_copy(alb[:, 0:ntc * HEADS], alf)

                    for j in range(cb):
                        lo0 = j * t2_lo
                        hi0 = ntl + j * t2_hi
                        # messages for the whole block in two batched ops
                        msg = p2m.tile([128, T2, F], bf16, tag="msg")
                        nc.vector.tensor_tensor(
                            msg[:, 0:t2_lo, :].rearrange("p t (h d) -> p t h d", h=HEADS),
                            g[:, lo0:lo0 + t2_lo, 0:F].rearrange(
                                "p t (h d) -> p t h d", h=HEADS),
                            alb[:, lo0 * HEADS:(lo0 + t2_lo) * HEADS].rearrange(
                                "p (t h) -> p t h", h=HEADS).unsqueeze(3)
                                .broadcast_to([128, t2_lo, HEADS, OUT_DIM]),
                            op=Alu.mult,
                        )
                        if t2_hi:
                            nc.vector.tensor_tensor(
                                msg[:, t2_lo:T2, :].rearrange(
                                    "p t (h d) -> p t h d", h=HEADS),
                                g[:, hi0:hi0 + t2_hi, 0:F].rearrange(
                                    "p t (h d) -> p t h d", h=HEADS),
                                alb[:, hi0 * HEADS:(hi0 + t2_hi) * HEADS].rearrange(
                                    "p (t h) -> p t h", h=HEADS).unsqueeze(3)
                                    .broadcast_to([128, t2_hi, HEADS, OUT_DIM]),
                                op=Alu.mult,
                            )
                        ps2 = p2ps.tile([128, F], dt.float32, tag="ps2")
                        for t in range(T2):
                            ti = lo0 + t if t < t2_lo else hi0 + (t - t2_lo)
                            nc.tensor.matmul(
                                ps2[:], eq[:, ti, :], msg[:, t, :],
                                start=(t == 0), stop=(t == T2 - 1),
                            )
                        osb = p2m.tile([128, F], dt.float32, tag="osb")
                        nc.vector.tensor_tensor(
                            osb[:], ps2[:], bias_bc[:], op=Alu.add
                        )
                        nc.sync.dma_start(
                            out[(c0 + j) * 128:(c0 + j + 1) * 128, :], osb[:]
                        )

    nc.compile()
    return nc


def _gat_forward(x, edges, W, a, bias, n_nodes, n_cores, run_opts=None):
    import ml_dtypes

    npad = _ceil_div(n_nodes, n_cores * 128) * n_cores * 128
    rpc = npad // n_cores
    nblk = rpc // 128

    src0 = edges[:, 0].astype(np.int64)
    dst0 = edges[:, 1].astype(np.int64)

    # --- node remap: sort by out-degree, interleave blocks across cores ---
    deg = np.bincount(src0, minlength=npad)
    order = np.argsort(deg, kind="stable")  # old ids, ascending degree
    blk_of = np.arange(npad) // 128
    core_of = blk_of % n_cores
    slot_of = blk_of // n_cores
    new_of_sortedpos = core_of * rpc + slot_of * 128 + (np.arange(npad) % 128)
    new_id = np.empty(npad, np.int64)
    new_id[order] = new_of_sortedpos
    old_of_new = np.empty(npad, np.int64)
    old_of_new[new_id] = np.arange(npad)

    src = new_id[src0]
    dst = new_id[dst0]

    k1_lo, k1_hi, p1_gidx, p1_vm = _build_p1_meta(src, dst, n_cores, nblk)
    t2_lo, t2_hi, chunks2, meta2 = _build_p2_meta(dst, src, n_cores, nblk, CB2)
    assert int(np.max(k1_lo + k1_hi)) <= K1MAX, (k1_lo.max(), k1_hi.max())

    nc = _build_bass_program(
        npad, rpc, nblk, k1_lo, k1_hi, t2_lo, t2_hi, chunks2, n_cores
    )

    x_pad = np.zeros((npad, IN_DIM), np.float32)
    x_pad[:n_nodes] = x
    x_perm = x_pad[old_of_new]
    xT = np.ascontiguousarray(x_perm.T).astype(ml_dtypes.bfloat16)
    W_b = np.ascontiguousarray(W.astype(np.float32)).astype(ml_dtypes.bfloat16)
    WT_b = np.ascontiguousarray(W_b.T)
    a_b = np.ascontiguousarray(a.astype(np.float32)).astype(ml_dtypes.bfloat16)

    in_maps = []
    for c in range(n_cores):
        g2, o2, r2 = meta2[c]
        in_maps.append({
            "xT": np.ascontiguousarray(xT[:, c * rpc:(c + 1) * rpc]),
            "W": W_b,
            "WT": WT_b,
            "a": a_b,
            "bias": np.ascontiguousarray(bias.astype(np.float32).reshape(1, F)),
            "p1_gidx": p1_gidx[c], "p1_vm": p1_vm[c],
            "p2_gidx": g2, "p2_oidx": o2, "p2_rel": r2,
        })

    from concourse.bass_utils import run_bass_kernel_spmd

    res = run_bass_kernel_spmd(
        nc, in_maps, core_ids=list(range(n_cores)), **(run_opts or {})
    )
    out_new = np.concatenate([r["out"] for r in res.results], axis=0)
    out_old = out_new[new_id]
    return out_old[:n_nodes], res


def kernel(x, edges, W, a, bias):
    x = np.asarray(x, np.float32)
    edges = np.asarray(edges)
    W = np.asarray(W, np.float32)
    a = np.asarray(a, np.float32)
    bias = np.asarray(bias, np.float32)
    out, _ = _gat_forward(x, edges, W, a, bias, N_NODES, N_CORES)
    return out


# revision 15
# speedup vs baseline: 2.5439x; 1.0441x over previous
"""GATConv (multi-head graph attention) on 8 Trainium2 NeuronCores.

kernel(**inputs) takes the FULL numpy inputs and returns the FULL
[50000, 256] float32 output.  All floating-point math runs on-device in a
Bass/Tile kernel; the host only does index bookkeeping (node remapping,
edge sorting, gather-index tables, padding) and dtype conversion.

Distribution: nodes are degree-sorted, then block-interleaved across the
8 cores (so the 8 cores' r-th blocks have near-identical degree
profiles and share compile-time tile counts).  Per core:
 - phase 0 projects its node slice (x @ [W | W@A], bf16) -> h + attention
   dot-products s.
 - phase 1 (degree-major): lanes = source nodes, free slots = their
   out-edges; one dma_gather stream fetches s_dst rows; s_src is a
   per-lane broadcast of the resident s table; denominators come from a
   free-dim reduce.  No selector matmuls, no own-node gathers.
 - phase 2 (scatter-form): edges grouped by dst block; one combined
   768B-row gather (h | s_src | recip) + an own-node s_dst gather; the
   one-hot scatter matrices are built on-chip from 2-byte rel indices
   and applied on the PE.
The big h AllGather overlaps phase 1; reciprocals arrive afterwards via
a tiny compact AllGather plus strided DRAM->DRAM writes into the local
combined-row table.
"""

import sys

sys.path.insert(0, "/opt/trn_rl_repo")

import numpy as np

N_NODES = 50000
N_EDGES = 800000
IN_DIM = 512
HEADS = 4
OUT_DIM = 64
F = HEADS * OUT_DIM  # 256
N_CORES = 8
HALF = 32768  # int16 gather index range split
GROW = 384  # G2 table bf16 elems/row: 256 h | 8 (s_src f32) | 8 (recip f32) | pad
SROW = 64  # SR table f32 elems/row: 4 s_src | 4 s_dst | pad -> 256B

CB2 = 2  # blocks per chunk, phase 2
MAXT = 24  # max tiles per dma_gather call, 256B rows
MAXTG = 16  # max tiles per call for 768B G2 rows
K1MAX = 48  # max slots per lane in phase 1 (tile allocation bound)


def _ceil_div(a, b):
    return (a + b - 1) // b


def _wrap16(arr_i16):
    """dma_gather idx layout: position i -> [i % 16, i // 16], x8 core groups."""
    n = arr_i16.shape[0]
    assert n % 16 == 0
    w = arr_i16.reshape(n // 16, 16).T
    return np.ascontiguousarray(np.tile(w, (8, 1)))


def _chunk_plan(nblk, cb_max):
    chunks = []
    c0 = 0
    while c0 < nblk:
        cb = min(cb_max, nblk - c0)
        chunks.append((c0, cb))
        c0 += cb
    return chunks


def _splits(t0, nt, maxt):
    out = []
    while nt > 0:
        n = min(nt, maxt)
        out.append((t0, n))
        t0 += n
        nt -= n
    return out


def _build_p1_meta(src, dst, n_cores, nblk):
    """Degree-major phase-1 layout.

    Per (core, block r): lanes = the block's 128 nodes; slot stream =
    [K_lo[r] low-half slots | K_hi[r] high-half slots] x 128 lanes,
    position (t*128+p) = slot t of lane p.  idx = dst (half-local),
    pad slots idx 0 (gather row 0; contribution masked by vmask).

    Returns (K_lo[r], K_hi[r], gidx[c] [128, ntot*8] i16,
             vmask[c] [128, ntot] f32  (1 real / 0 pad))
    """
    rpc = nblk * 128
    hi = (dst >= HALF).astype(np.int64)
    order = np.lexsort((dst, hi, src))
    ss, ds, hs = src[order], dst[order], hi[order]
    nh = ss * 2 + hs
    counts = np.bincount(nh, minlength=n_cores * rpc * 2)
    starts = np.zeros(len(counts), np.int64)
    np.cumsum(counts[:-1], out=starts[1:])
    rank = np.arange(len(ss), dtype=np.int64) - starts[nh]

    cnt2 = counts.reshape(n_cores, nblk, 128, 2)
    K_lo = cnt2[:, :, :, 0].max(axis=(0, 2))  # [nblk] max over cores & lanes
    K_hi = cnt2[:, :, :, 1].max(axis=(0, 2))
    K_lo = np.maximum(K_lo, 1)
    K = K_lo + K_hi
    base = np.zeros(nblk + 1, np.int64)
    np.cumsum(K, out=base[1:])
    ntot = int(base[-1])  # slot-tiles per core

    core = ss // rpc
    blk = (ss % rpc) >> 7
    lane = ss & 127
    slot = np.where(hs == 0, rank, K_lo[blk] + rank)
    pos = (core * ntot + base[blk] + slot) * 128 + lane

    total = n_cores * ntot * 128
    gidx = np.zeros(total, np.int16)
    gidx[pos] = (ds - hs * HALF).astype(np.int16)
    vm = np.zeros(total, np.float32)
    vm[pos] = 1.0

    per = ntot * 128
    gidx_c, vm_c = [], []
    for c in range(n_cores):
        sl = slice(c * per, (c + 1) * per)
        gidx_c.append(_wrap16(gidx[sl]))
        vm_c.append(np.ascontiguousarray(vm[sl].reshape(ntot, 128).T))
    return K_lo.astype(int), K_hi.astype(int), gidx_c, vm_c


def _build_p2_meta(key, other, n_cores, nblk, cb_max):
    """Phase-2 (scatter-form) meta: chunked tile layout.

    Edges grouped by key block; chunk of cb blocks lays tiles out as
    [cb*T_lo low tiles | cb*T_hi high tiles]; block j's low tiles start
    at j*T_lo, its high tiles at cb*T_lo + j*T_hi.
    """
    rpc = nblk * 128
    gblk = (key >> 7).astype(np.int64)
    hi = (other >= HALF).astype(np.int64)
    order = np.lexsort((other, hi, gblk))
    kb, ob, hb, gb = key[order], other[order], hi[order], gblk[order]

    nrun = n_cores * nblk * 2
    run = gb * 2 + hb
    counts = np.bincount(run, minlength=nrun)
    T_lo = max(1, int(np.max(_ceil_div(counts[0::2], 128))))
    T_hi = int(np.max(_ceil_div(counts[1::2], 128)))
    T = T_lo + T_hi

    chunks = _chunk_plan(nblk, cb_max)
    ntiles = nblk * T
    tb = np.zeros(len(chunks) + 1, np.int64)
    for i, (_, cb_) in enumerate(chunks):
        tb[i + 1] = tb[i] + cb_ * T

    starts = np.zeros(nrun, np.int64)
    np.cumsum(counts[:-1], out=starts[1:])
    rank = np.arange(len(kb), dtype=np.int64) - starts[run]
    blocal = gb % nblk
    core = gb // nblk
    chunk_id = blocal // cb_max
    c0_arr = chunk_id * cb_max
    cb_arr = np.minimum(cb_max, nblk - c0_arr)
    tic = np.where(
        hb == 0,
        (blocal - c0_arr) * T_lo + rank // 128,
        cb_arr * T_lo + (blocal - c0_arr) * T_hi + rank // 128,
    )
    tile = tb[chunk_id] + tic
    pos = (core * ntiles + tile) * 128 + rank % 128

    total = n_cores * ntiles * 128
    gidx = np.zeros(total, np.int16)
    oidx = np.zeros(total, np.int16)
    rel = np.full(total, 128, np.int16)
    gidx[pos] = (ob - hb * HALF).astype(np.int16)
    oidx[pos] = (kb - core * rpc).astype(np.int16)
    rel[pos] = (kb & 127).astype(np.int16)

    per_core = []
    per = ntiles * 128
    for c in range(n_cores):
        sl = slice(c * per, (c + 1) * per)
        per_core.append((
            _wrap16(gidx[sl]),
            _wrap16(oidx[sl]),
            np.ascontiguousarray(rel[sl].reshape(ntiles, 128).T.astype(np.float32)),
        ))
    return T_lo, T_hi, chunks, per_core


def _build_bass_program(npad, rpc, nblk, k1_lo, k1_hi, t2_lo, t2_hi, chunks2,
                        n_cores, enable_asserts=False):
    import concourse.bacc as bacc
    import concourse.mybir as mybir
    import concourse.tile as tile

    dt = mybir.dt
    Alu = mybir.AluOpType
    Act = mybir.ActivationFunctionType
    T2 = t2_lo + t2_hi
    nt2 = nblk * T2
    K1 = [int(k1_lo[r] + k1_hi[r]) for r in range(nblk)]
    base1 = [0]
    for r in range(nblk):
        base1.append(base1[-1] + K1[r])
    nt1 = base1[-1]
    KC = IN_DIM // 128
    WCOL = F + 2 * HEADS  # 264
    H2 = 2 * HEADS
    bf16 = dt.bfloat16

    nc = bacc.Bacc(
        "TRN2",
        target_bir_lowering=False,
        debug=False,
        enable_asserts=enable_asserts,
        num_devices=n_cores,
        num_swdge_queues=4,
    )

    xT = nc.dram_tensor("xT", [IN_DIM, rpc], bf16, kind="ExternalInput")
    W_in = nc.dram_tensor("W", [IN_DIM, F], bf16, kind="ExternalInput")
    WT_in = nc.dram_tensor("WT", [F, IN_DIM], bf16, kind="ExternalInput")
    a_in = nc.dram_tensor("a", [HEADS, 2 * OUT_DIM], bf16, kind="ExternalInput")
    bias_in = nc.dram_tensor("bias", [1, F], dt.float32, kind="ExternalInput")
    p1_gidx = nc.dram_tensor("p1_gidx", [128, nt1 * 8], dt.int16, kind="ExternalInput")
    p1_vm = nc.dram_tensor("p1_vm", [128, nt1], dt.float32, kind="ExternalInput")
    p2_gidx = nc.dram_tensor("p2_gidx", [128, nt2 * 8], dt.int16, kind="ExternalInput")
    p2_oidx = nc.dram_tensor("p2_oidx", [128, nt2 * 8], dt.int16, kind="ExternalInput")
    p2_rel = nc.dram_tensor("p2_rel", [128, nt2], dt.float32, kind="ExternalInput")
    out = nc.dram_tensor("out", [rpc, F], dt.float32, kind="ExternalOutput")

    with tile.TileContext(nc) as tc:
        with (
            tc.tile_pool(name="const", bufs=1) as cpool,
            tc.tile_pool(name="dram", bufs=1, space="DRAM") as dram,
        ):
            H_own = dram.tile([rpc, F], bf16)
            H_full = dram.tile([npad, F], bf16, addr_space="Shared")
            S_ownE = dram.tile([rpc, SROW], dt.float32)
            S_own8 = dram.tile([rpc, 8], dt.float32)
            S_all8 = dram.tile([npad, 8], dt.float32, addr_space="Shared")
            SR_full = dram.tile([npad, SROW], dt.float32)
            G2 = dram.tile([npad, GROW], bf16)
            R_own = dram.tile([rpc, HEADS], dt.float32)
            R_all = dram.tile([npad, HEADS], dt.float32, addr_space="Shared")

            # ---------------- constants ----------------
            iota_i = cpool.tile([128, 128], dt.int32)
            nc.gpsimd.iota(iota_i[:], pattern=[[1, 128]], channel_multiplier=0)
            iota_f = cpool.tile([128, 128], dt.float32)
            nc.vector.tensor_copy(iota_f[:], iota_i[:])

            bias_bc = cpool.tile([128, F], dt.float32)
            nc.sync.dma_start(bias_bc[:1, :], bias_in[:, :])
            nc.gpsimd.partition_broadcast(bias_bc[:], bias_bc[:1, :])

            A0 = cpool.tile([128, H2], bf16)
            A1 = cpool.tile([128, H2], bf16)
            nc.vector.memset(A0[:], 0.0)
            nc.vector.memset(A1[:], 0.0)
            for h in range(HEADS):
                dstA = A0 if h < 2 else A1
                p0c = (h % 2) * OUT_DIM
                nc.sync.dma_start(dstA[p0c:p0c + OUT_DIM, h:h + 1], a_in[h:h + 1, 0:OUT_DIM])
                nc.sync.dma_start(
                    dstA[p0c:p0c + OUT_DIM, HEADS + h:HEADS + h + 1],
                    a_in[h:h + 1, OUT_DIM:2 * OUT_DIM],
                )

            W_sb = cpool.tile([128, KC * WCOL], bf16)
            for kc in range(KC):
                nc.sync.dma_start(
                    W_sb[:, kc * WCOL:kc * WCOL + F], W_in[kc * 128:(kc + 1) * 128, :]
                )
            WT0 = cpool.tile([128, IN_DIM], bf16)
            WT1 = cpool.tile([128, IN_DIM], bf16)
            nc.sync.dma_start(WT0[:], WT_in[0:128, :])
            nc.sync.dma_start(WT1[:], WT_in[128:256, :])
            with tc.tile_pool(name="psum_pre", bufs=2, space="PSUM") as pp:
                for kc in range(KC):
                    pwa = pp.tile([128, H2], dt.float32, tag="wa")
                    for fc in range(2):
                        wt = WT0 if fc == 0 else WT1
                        A = A0 if fc == 0 else A1
                        nc.tensor.matmul(
                            pwa[:], wt[:, kc * 128:(kc + 1) * 128], A[:],
                            start=(fc == 0), stop=(fc == 1),
                        )
                    nc.vector.tensor_copy(W_sb[:, kc * WCOL + F:(kc + 1) * WCOL], pwa[:])

            # s_src of own nodes, resident (per-lane broadcast source in p1)
            s_res = cpool.tile([128, nblk * HEADS], dt.float32)
            rec_all = cpool.tile([128, nblk * HEADS], dt.float32)

            # ---------------- phase 0: projection ----------------
            with (
                tc.tile_pool(name="p0x", bufs=1) as p0x,
                tc.tile_pool(name="p0", bufs=3) as p0pool,
                tc.tile_pool(name="p0ps", bufs=2, space="PSUM") as p0ps,
            ):
                xres = p0x.tile([128, KC * rpc], bf16)
                for kc in range(KC):
                    nc.sync.dma_start(
                        xres[:, kc * rpc:(kc + 1) * rpc],
                        xT[kc * 128:(kc + 1) * 128, :],
                    )
                for r in range(nblk):
                    ps = p0ps.tile([128, WCOL], dt.float32, tag="hps")
                    for kc in range(KC):
                        nc.tensor.matmul(
                            ps[:], xres[:, kc * rpc + r * 128:kc * rpc + (r + 1) * 128],
                            W_sb[:, kc * WCOL:(kc + 1) * WCOL],
                            start=(kc == 0), stop=(kc == KC - 1),
                        )
                    gsb = p0pool.tile([128, F], bf16, tag="gsb")
                    nc.vector.tensor_copy(gsb[:], ps[:, 0:F])
                    nc.sync.dma_start(H_own[r * 128:(r + 1) * 128, :], gsb[:])
                    ssb = p0pool.tile([128, 8], dt.float32, tag="ssb")
                    nc.vector.tensor_copy(ssb[:], ps[:, F:WCOL])
                    nc.sync.dma_start(S_ownE[r * 128:(r + 1) * 128, 0:8], ssb[:])
                    nc.sync.dma_start(S_own8[r * 128:(r + 1) * 128, :], ssb[:])
                    nc.vector.tensor_copy(
                        s_res[:, r * HEADS:(r + 1) * HEADS], ps[:, F:F + HEADS]
                    )

            # ---------------- collectives after phase 0 ----------------
            nc.gpsimd.collective_compute(
                "AllGather", Alu.bypass,
                ins=[S_own8[:].opt()], outs=[S_all8[:].opt()],
                replica_groups=[list(range(n_cores))],
            )
            nc.gpsimd.collective_compute(
                "AllGather", Alu.bypass,
                ins=[H_own[:].opt()], outs=[H_full[:].opt()],
                replica_groups=[list(range(n_cores))],
            )
            # expand compact s into the 256B-stride gather table (for p1)
            nc.sync.dma_start(SR_full[:, 0:8], S_all8[:, :])
            # assemble the combined phase-2 row table G2 (h part + s_src part);
            # consumed only by phase 2, so these overlap phase 1
            nc.sync.dma_start(G2[:, 0:F], H_full[:, :])
            nc.sync.dma_start(G2[:, F:F + H2].bitcast(dt.float32), S_all8[:, 0:4])

            # ---------------- phase 1: denominators (degree-major) ----------
            with (
                tc.tile_pool(name="p1i", bufs=6) as p1i,
                tc.tile_pool(name="p1", bufs=4) as p1pool,
                tc.tile_pool(name="p1v", bufs=6) as p1v,
            ):
                qn = 0
                for r in range(nblk):
                    klo = int(k1_lo[r])
                    khi = int(k1_hi[r])
                    kk = klo + khi
                    b0 = base1[r]
                    gix = p1i.tile([128, K1MAX * 8], dt.int16, tag="gix")
                    nc.sync.dma_start(
                        gix[:, 0:kk * 8], p1_gidx[:, b0 * 8:(b0 + kk) * 8]
                    )
                    vms = p1i.tile([128, K1MAX], dt.float32, tag="vm")
                    nc.sync.dma_start(vms[:, 0:kk], p1_vm[:, b0:b0 + kk])

                    g1 = p1pool.tile([128, K1MAX, SROW], dt.float32, tag="g1")
                    for t0, n in _splits(0, klo, MAXT):
                        nc.gpsimd.dma_gather(
                            g1[:, t0:t0 + n, :], SR_full[0:HALF, :],
                            gix[:, t0 * 8:(t0 + n) * 8], n * 128, n * 128, SROW,
                            single_packet=False, queue_num=qn % 4,
                        )
                        qn += 1
                    for t0, n in _splits(klo, khi, MAXT):
                        nc.gpsimd.dma_gather(
                            g1[:, t0:t0 + n, :], SR_full[HALF:npad, :],
                            gix[:, t0 * 8:(t0 + n) * 8], n * 128, n * 128, SROW,
                            single_packet=False, queue_num=qn % 4,
                        )
                        qn += 1

                    # z[p, k, h] = s_res[p, r, h] + s_dst[p, k, h]
                    z = p1v.tile([128, K1MAX, HEADS], dt.float32, tag="z")
                    nc.vector.tensor_tensor(
                        z[:, 0:kk, :],
                        g1[:, 0:kk, HEADS:H2],
                        s_res[:, r * HEADS:(r + 1) * HEADS].unsqueeze(1)
                            .broadcast_to([128, kk, HEADS]),
                        op=Alu.add,
                    )
                    zf = z[:, 0:kk, :].rearrange("p t h -> p (t h)")
                    nc.vector.scalar_tensor_tensor(
                        zf, zf, 0.2, zf, op0=Alu.mult, op1=Alu.max
                    )
                    nc.scalar.activation(zf, zf, Act.Exp)
                    # mask pads, then reduce over slots
                    nc.vector.tensor_tensor(
                        z[:, 0:kk, :], z[:, 0:kk, :],
                        vms[:, 0:kk].unsqueeze(2).broadcast_to([128, kk, HEADS]),
                        op=Alu.mult,
                    )
                    nc.vector.tensor_reduce(
                        rec_all[:, r * HEADS:(r + 1) * HEADS],
                        z[:, 0:kk, :].rearrange("p t h -> p h t"),
                        axis=mybir.AxisListType.X, op=Alu.add,
                    )
                recd = cpool.tile([128, nblk * HEADS], dt.float32)
                nc.vector.tensor_scalar_add(rec_all[:], rec_all[:], 1e-10)
                nc.vector.reciprocal(recd[:], rec_all[:])
                nc.sync.dma_start(
                    R_own[:].rearrange("(b p) h -> p b h", p=128),
                    recd[:].rearrange("p (b h) -> p b h", h=HEADS),
                )

            nc.gpsimd.collective_compute(
                "AllGather", Alu.bypass,
                ins=[R_own[:].opt()], outs=[R_all[:].opt()],
                replica_groups=[list(range(n_cores))],
            )
            # write reciprocals into the recip slot of every G2 row
            nc.sync.dma_start(G2[:, F + H2:F + 2 * H2].bitcast(dt.float32), R_all[:, :])

            # ---------------- phase 2: aggregate messages ----------------
            with (
                tc.tile_pool(name="p2i", bufs=4) as p2i,
                tc.tile_pool(name="p2", bufs=3) as p2pool,
                tc.tile_pool(name="p2eq", bufs=2) as p2eq,
                tc.tile_pool(name="p2v", bufs=4) as p2v,
                tc.tile_pool(name="p2m", bufs=3) as p2m,
                tc.tile_pool(name="p2ps", bufs=2, space="PSUM") as p2ps,
            ):
                qn = 0
                for ci, (c0, cb) in enumerate(chunks2):
                    tg0 = 0
                    for cc0, ccb in chunks2[:ci]:
                        tg0 += ccb * T2
                    ntl = cb * t2_lo
                    nth = cb * t2_hi
                    ntc = cb * T2
                    gix = p2i.tile([128, CB2 * T2 * 8], dt.int16, tag="gix")
                    nc.sync.dma_start(
                        gix[:, 0:ntc * 8], p2_gidx[:, tg0 * 8:(tg0 + ntc) * 8]
                    )
                    oix = p2i.tile([128, CB2 * T2 * 8], dt.int16, tag="oix")
                    nc.sync.dma_start(
                        oix[:, 0:ntc * 8], p2_oidx[:, tg0 * 8:(tg0 + ntc) * 8]
                    )
                    rels = p2i.tile([128, CB2 * T2], dt.float32, tag="rel")
                    nc.sync.dma_start(rels[:, 0:ntc], p2_rel[:, tg0:tg0 + ntc])

                    g = p2pool.tile([128, CB2 * T2, GROW], bf16, tag="g")
                    own = p2pool.tile([128, CB2 * T2, SROW], dt.float32, tag="own")
                    for t0, n in _splits(0, ntl, MAXTG):
                        nc.gpsimd.dma_gather(
                            g[:, t0:t0 + n, :], G2[0:HALF, :],
                            gix[:, t0 * 8:(t0 + n) * 8], n * 128, n * 128, GROW,
                            single_packet=False, queue_num=qn % 4,
                        )
                        qn += 1
                    for t0, n in _splits(ntl, nth, MAXTG):
                        nc.gpsimd.dma_gather(
                            g[:, t0:t0 + n, :], G2[HALF:npad, :],
                            gix[:, t0 * 8:(t0 + n) * 8], n * 128, n * 128, GROW,
                            single_packet=False, queue_num=qn % 4,
                        )
                        qn += 1
                    for t0, n in _splits(0, ntc, MAXT):
                        nc.gpsimd.dma_gather(
                            own[:, t0:t0 + n, :], S_ownE[:, :],
                            oix[:, t0 * 8:(t0 + n) * 8], n * 128, n * 128, SROW,
                            single_packet=False, queue_num=qn % 4,
                        )
                        qn += 1

                    # build all one-hot tiles of the chunk in one DVE op
                    eq = p2eq.tile([128, CB2 * T2, 128], bf16, tag="eq")
                    nc.vector.tensor_tensor(
                        eq[:, 0:ntc, :],
                        iota_f[:].unsqueeze(1).broadcast_to([128, ntc, 128]),
                        rels[:, 0:ntc].unsqueeze(2).broadcast_to([128, ntc, 128]),
                        op=Alu.is_equal,
                    )

                    # alpha chain, whole chunk at once
                    al = p2v.tile([128, CB2 * T2, HEADS], dt.float32, tag="al")
                    nc.vector.tensor_tensor(
                        al[:, 0:ntc, :],
                        g[:, 0:ntc, F:F + H2].bitcast(dt.float32),
                        own[:, 0:ntc, HEADS:H2], op=Alu.add,
                    )
                    alf = al[:, 0:ntc, :].rearrange("p t h -> p (t h)")
                    nc.vector.scalar_tensor_tensor(
                        alf, alf, 0.2, alf, op0=Alu.mult, op1=Alu.max
                    )
                    nc.scalar.activation(alf, alf, Act.Exp)
                    nc.vector.tensor_tensor(
                        al[:, 0:ntc, :], al[:, 0:ntc, :],
                        g[:, 0:ntc, F + H2:F + 2 * H2].bitcast(dt.float32),
                        op=Alu.mult,
                    )
                    alb = p2v.tile([128, CB2 * T2 * HEADS], bf16, tag="alb")
                    nc.vector.tensor_copy(alb[:, 0:ntc * HEADS], alf)

                    for j in range(cb):
                        lo0 = j * t2_lo
                        hi0 = ntl + j * t2_hi
                        # messages for the whole block in two batched ops
                        msg = p2m.tile([128, T2, F], bf16, tag="msg")
                        nc.vector.tensor_tensor(
                            msg[:, 0:t2_lo, :].rearrange("p t (h d) -> p t h d", h=HEADS),
                            g[:, lo0:lo0 + t2_lo, 0:F].rearrange(
                                "p t (h d) -> p t h d", h=HEADS),
                            alb[:, lo0 * HEADS:(lo0 + t2_lo) * HEADS].rearrange(
                                "p (t h) -> p t h", h=HEADS).unsqueeze(3)
                                .broadcast_to([128, t2_lo, HEADS, OUT_DIM]),
                            op=Alu.mult,
                        )
                        if t2_hi:
                            nc.vector.tensor_tensor(
                                msg[:, t2_lo:T2, :].rearrange(
                                    "p t (h d) -> p t h d", h=HEADS),
                                g[:, hi0:hi0 + t2_hi, 0:F].rearrange(
                                    "p t (h d) -> p t h d", h=HEADS),
                                alb[:, hi0 * HEADS:(hi0 + t2_hi) * HEADS].rearrange(
                                    "p (t h) -> p t h", h=HEADS).unsqueeze(3)
                                    .broadcast_to([128, t2_hi, HEADS, OUT_DIM]),
                                op=Alu.mult,
                            )
                        ps2 = p2ps.tile([128, F], dt.float32, tag="ps2")
                        for t in range(T2):
                            ti = lo0 + t if t < t2_lo else hi0 + (t - t2_lo)
                            nc.tensor.matmul(
                                ps2[:], eq[:, ti, :], msg[:, t, :],
                                start=(t == 0), stop=(t == T2 - 1),
                            )
                        osb = p2m.tile([128, F], dt.float32, tag="osb")
                        nc.vector.tensor_tensor(
                            osb[:], ps2[:], bias_bc[:], op=Alu.add
                        )
                        nc.sync.dma_start(
                            out[(c0 + j) * 128:(c0 + j + 1) * 128, :], osb[:]
                        )

    nc.compile()
    return nc


def _gat_forward(x, edges, W, a, bias, n_nodes, n_cores, run_opts=None):
    import ml_dtypes

    npad = _ceil_div(n_nodes, n_cores * 128) * n_cores * 128
    rpc = npad // n_cores
    nblk = rpc // 128

    src0 = edges[:, 0].astype(np.int64)
    dst0 = edges[:, 1].astype(np.int64)

    # --- node remap: sort by out-degree, interleave blocks across cores ---
    deg = np.bincount(src0, minlength=npad)
    order = np.argsort(deg, kind="stable")  # old ids, ascending degree
    blk_of = np.arange(npad) // 128
    core_of = blk_of % n_cores
    slot_of = blk_of // n_cores
    new_of_sortedpos = core_of * rpc + slot_of * 128 + (np.arange(npad) % 128)
    new_id = np.empty(npad, np.int64)
    new_id[order] = new_of_sortedpos
    old_of_new = np.empty(npad, np.int64)
    old_of_new[new_id] = np.arange(npad)

    src = new_id[src0]
    dst = new_id[dst0]

    k1_lo, k1_hi, p1_gidx, p1_vm = _build_p1_meta(src, dst, n_cores, nblk)
    t2_lo, t2_hi, chunks2, meta2 = _build_p2_meta(dst, src, n_cores, nblk, CB2)
    assert int(np.max(k1_lo + k1_hi)) <= K1MAX, (k1_lo.max(), k1_hi.max())

    nc = _build_bass_program(
        npad, rpc, nblk, k1_lo, k1_hi, t2_lo, t2_hi, chunks2, n_cores
    )

    x_pad = np.zeros((npad, IN_DIM), np.float32)
    x_pad[:n_nodes] = x
    x_perm = x_pad[old_of_new]
    xT = np.ascontiguousarray(x_perm.T).astype(ml_dtypes.bfloat16)
    W_b = np.ascontiguousarray(W.astype(np.float32)).astype(ml_dtypes.bfloat16)
    WT_b = np.ascontiguousarray(W_b.T)
    a_b = np.ascontiguousarray(a.astype(np.float32)).astype(ml_dtypes.bfloat16)

    in_maps = []
    for c in range(n_cores):
        g2, o2, r2 = meta2[c]
        in_maps.append({
            "xT": np.ascontiguousarray(xT[:, c * rpc:(c + 1) * rpc]),
            "W": W_b,
            "WT": WT_b,
            "a": a_b,
            "bias": np.ascontiguousarray(bias.astype(np.float32).reshape(1, F)),
            "p1_gidx": p1_gidx[c], "p1_vm": p1_vm[c],
            "p2_gidx": g2, "p2_oidx": o2, "p2_rel": r2,
        })

    from concourse.bass_utils import run_bass_kernel_spmd

    res = run_bass_kernel_spmd(
        nc, in_maps, core_ids=list(range(n_cores)), **(run_opts or {})
    )
    out_new = np.concatenate([r["out"] for r in res.results], axis=0)
    out_old = out_new[new_id]
    return out_old[:n_nodes], res


def kernel(x, edges, W, a, bias):
    x = np.asarray(x, np.float32)
    edges = np.asarray(edges)
    W = np.asarray(W, np.float32)
    a = np.asarray(a, np.float32)
    bias = np.asarray(bias, np.float32)
    out, _ = _gat_forward(x, edges, W, a, bias, N_NODES, N_CORES)
    return out


# revision 16
# speedup vs baseline: 2.8958x; 1.1383x over previous
"""GATConv (multi-head graph attention) on 8 Trainium2 NeuronCores.

kernel(**inputs) takes the FULL numpy inputs and returns the FULL
[50000, 256] float32 output.  All floating-point math runs on-device in a
Bass/Tile kernel; the host only does index bookkeeping (edge sorting,
gather-index tables, 0/1 selector matrices) and shape padding.

Distribution: nodes are block-partitioned across the 8 cores.  Per core:
phase 0 projects its node slice (x @ [W | W@A] via fp32r matmuls) producing
h and the attention dot-products s; phase 1 computes softmax denominators
for its source nodes (random dma_gather of s rows + selector matmuls);
phase 2 aggregates messages at its destination nodes (random dma_gather of
[h|s|recip] rows, alpha on DVE/ACT, one selector matmul per 128-edge tile
accumulating into PSUM).  Two AllGathers replicate the per-node tables
between phases.
"""

import sys

sys.path.insert(0, "/opt/trn_rl_repo")

import numpy as np

N_NODES = 50000
N_EDGES = 800000
IN_DIM = 512
HEADS = 4
OUT_DIM = 64
F = HEADS * OUT_DIM  # 256
N_CORES = 8
HALF = 32768  # int16 gather index range split
GROW = 384  # G table bf16 elems/row: 256 h | 8 (s_src f32) | 8 (recip f32) | pad
SROW = 64  # S table f32 elems/row: 4 s_src | 4 s_dst | pad  -> 256B


def _ceil_div(a, b):
    return (a + b - 1) // b


def _wrap16(arr_i16):
    """dma_gather idx layout: position i -> [i % 16, i // 16], x8 core groups."""
    n = arr_i16.shape[0]
    assert n % 16 == 0
    w = arr_i16.reshape(n // 16, 16).T
    return np.ascontiguousarray(np.tile(w, (8, 1)))


def _build_phase_meta(key, other, rel, n_cores, nblk):
    """Per-core gather indices + bf16 selector matrices for one edge pass.

    Edges are grouped by key-node block (128 nodes); within a block a low
    run (other < HALF) and a high run, each padded to a multiple of 128.
    Tile counts (T_lo, T_hi) are uniform across blocks/cores (same program).

    Returns (T_lo, T_hi, gidx[c], sidx[c], eq[c]):
      gidx: [128, nblk*T*8] int16  gather idx into half table (pad -> 0)
      sidx: [128, nblk*T*8] int16  key_rel window-gather idx (pad -> 0)
      eq:   [nblk*T*128, 128] bf16 selector: eq[lane, j] = (key_rel == j)
    """
    import ml_dtypes

    gblk = key >> 7
    hi = (other >= HALF).astype(np.int64)
    run = gblk * 2 + hi
    order = np.lexsort((other, run))
    run_s = run[order]
    other_s = other[order]
    rel_s = rel[order]

    nruns = n_cores * nblk * 2
    counts = np.bincount(run_s, minlength=nruns)
    T_lo = max(1, int(np.max(_ceil_div(counts[0::2], 128))))
    T_hi = int(np.max(_ceil_div(counts[1::2], 128)))
    T = T_lo + T_hi

    starts = np.zeros(nruns, np.int64)
    np.cumsum(counts[:-1], out=starts[1:])
    rank = np.arange(len(key), dtype=np.int64) - starts[run_s]
    pos = (run_s >> 1) * (T * 128) + hi[order] * (T_lo * 128) + rank

    total = n_cores * nblk * T * 128
    gidx = np.zeros(total, np.int16)
    sidx = np.zeros(total, np.int16)
    gidx[pos] = (other_s - (other_s >= HALF) * HALF).astype(np.int16)
    sidx[pos] = rel_s.astype(np.int16)
    eq = np.zeros((total, 128), ml_dtypes.bfloat16)
    eq[pos, rel_s] = 1.0

    per_core = nblk * T * 128
    gidx_c, sidx_c, eq_c = [], [], []
    for c in range(n_cores):
        sl = slice(c * per_core, (c + 1) * per_core)
        gidx_c.append(_wrap16(gidx[sl]))
        sidx_c.append(_wrap16(sidx[sl]))
        eq_c.append(np.ascontiguousarray(eq[sl]))
    return T_lo, T_hi, gidx_c, sidx_c, eq_c


def _build_bass_program(npad, rpc, nblk, t1_lo, t1_hi, t2_lo, t2_hi, n_cores,
                        enable_asserts=False):
    import concourse.bacc as bacc
    import concourse.mybir as mybir
    import concourse.tile as tile

    dt = mybir.dt
    Alu = mybir.AluOpType
    Act = mybir.ActivationFunctionType
    T1 = t1_lo + t1_hi
    T2 = t2_lo + t2_hi
    KC = IN_DIM // 128
    WCOL = F + 2 * HEADS  # 264
    H2 = 2 * HEADS
    f32r = dt.float32r
    bf16 = dt.bfloat16

    nc = bacc.Bacc(
        "TRN2",
        target_bir_lowering=False,
        debug=False,
        enable_asserts=enable_asserts,
        num_devices=n_cores,
        num_swdge_queues=4,
    )

    xT = nc.dram_tensor("xT", [IN_DIM, rpc], dt.float32, kind="ExternalInput")
    W_in = nc.dram_tensor("W", [IN_DIM, F], dt.float32, kind="ExternalInput")
    a_in = nc.dram_tensor("a", [HEADS, 2 * OUT_DIM], dt.float32, kind="ExternalInput")
    bias_in = nc.dram_tensor("bias", [1, F], dt.float32, kind="ExternalInput")
    p1_gidx = nc.dram_tensor("p1_gidx", [128, nblk * T1 * 8], dt.int16, kind="ExternalInput")
    p1_sidx = nc.dram_tensor("p1_sidx", [128, nblk * T1 * 8], dt.int16, kind="ExternalInput")
    p1_eq = nc.dram_tensor("p1_eq", [nblk * T1 * 128, 128], bf16, kind="ExternalInput")
    p2_gidx = nc.dram_tensor("p2_gidx", [128, nblk * T2 * 8], dt.int16, kind="ExternalInput")
    p2_sidx = nc.dram_tensor("p2_sidx", [128, nblk * T2 * 8], dt.int16, kind="ExternalInput")
    p2_eq = nc.dram_tensor("p2_eq", [nblk * T2 * 128, 128], bf16, kind="ExternalInput")
    out = nc.dram_tensor("out", [rpc, F], dt.float32, kind="ExternalOutput")

    with tile.TileContext(nc) as tc:
        with (
            tc.tile_pool(name="const", bufs=1) as cpool,
            tc.tile_pool(name="dram", bufs=1, space="DRAM") as dram,
        ):
            G_own = dram.tile([rpc, GROW], bf16)
            S_own = dram.tile([rpc, SROW], dt.float32)
            G_full = dram.tile([npad, GROW], bf16, addr_space="Shared")
            S_full = dram.tile([npad, SROW], dt.float32, addr_space="Shared")

            # ---------------- constants ----------------
            iota_i = cpool.tile([128, 128], dt.int32)
            nc.gpsimd.iota(iota_i[:], pattern=[[1, 128]], channel_multiplier=0)
            iota_f = cpool.tile([128, 128], dt.float32)
            nc.vector.tensor_copy(iota_f[:], iota_i[:])
            pidx_i = cpool.tile([128, 1], dt.int32)
            nc.gpsimd.iota(pidx_i[:], pattern=[[0, 1]], channel_multiplier=1)
            pidx_f = cpool.tile([128, 1], dt.float32)
            nc.vector.tensor_copy(pidx_f[:], pidx_i[:])
            ident = cpool.tile([128, 128], dt.float32)
            nc.vector.tensor_scalar(ident[:], iota_f[:], pidx_f[:], None, op0=Alu.is_equal)

            bias_bc = cpool.tile([128, F], dt.float32)
            nc.sync.dma_start(bias_bc[:1, :], bias_in[:, :])
            nc.gpsimd.partition_broadcast(bias_bc[:], bias_bc[:1, :])

            A0 = cpool.tile([128, H2], dt.float32)
            A1 = cpool.tile([128, H2], dt.float32)
            nc.vector.memset(A0[:], 0.0)
            nc.vector.memset(A1[:], 0.0)
            for h in range(HEADS):
                dstA = A0 if h < 2 else A1
                p0 = (h % 2) * OUT_DIM
                nc.sync.dma_start(dstA[p0:p0 + OUT_DIM, h:h + 1], a_in[h:h + 1, 0:OUT_DIM])
                nc.sync.dma_start(
                    dstA[p0:p0 + OUT_DIM, HEADS + h:HEADS + h + 1],
                    a_in[h:h + 1, OUT_DIM:2 * OUT_DIM],
                )

            W_sb = cpool.tile([128, KC * WCOL], dt.float32)
            for kc in range(KC):
                nc.sync.dma_start(
                    W_sb[:, kc * WCOL:kc * WCOL + F], W_in[kc * 128:(kc + 1) * 128, :]
                )
            WT0 = cpool.tile([128, IN_DIM], dt.float32)
            WT1 = cpool.tile([128, IN_DIM], dt.float32)
            with tc.tile_pool(name="psum_pre", bufs=2, space="PSUM") as pp:
                for kc in range(KC):
                    for fc in range(2):
                        pt = pp.tile([128, 128], dt.float32, tag="tr")
                        nc.tensor.transpose(
                            pt[:],
                            W_sb[:, kc * WCOL + fc * 128:kc * WCOL + (fc + 1) * 128],
                            ident[:],
                        )
                        wt = WT0 if fc == 0 else WT1
                        nc.vector.tensor_copy(wt[:, kc * 128:(kc + 1) * 128], pt[:])
                for kc in range(KC):
                    pwa = pp.tile([128, H2], dt.float32, tag="wa")
                    for fc in range(2):
                        wt = WT0 if fc == 0 else WT1
                        A = A0 if fc == 0 else A1
                        nc.tensor.matmul(
                            pwa[:], wt[:, kc * 128:(kc + 1) * 128], A[:],
                            start=(fc == 0), stop=(fc == 1),
                        )
                    nc.vector.tensor_copy(W_sb[:, kc * WCOL + F:(kc + 1) * WCOL], pwa[:])
            W_sbr = cpool.tile([128, KC * WCOL], f32r)
            nc.vector.tensor_copy(W_sbr[:], W_sb[:])

            # s values of the core's own nodes, resident
            s_res = cpool.tile([128, nblk * H2], dt.float32)
            rec_all = cpool.tile([128, nblk * HEADS], dt.float32)

            # ---------------- phase 0: projection ----------------
            with (
                tc.tile_pool(name="p0", bufs=3) as p0pool,
                tc.tile_pool(name="p0ps", bufs=2, space="PSUM") as p0ps,
            ):
                for r in range(nblk):
                    xt = p0pool.tile([128, KC * 128], dt.float32, tag="xt")
                    for kc in range(KC):
                        nc.sync.dma_start(
                            xt[:, kc * 128:(kc + 1) * 128],
                            xT[kc * 128:(kc + 1) * 128, r * 128:(r + 1) * 128],
                        )
                    xtr = p0pool.tile([128, KC * 128], f32r, tag="xtr")
                    nc.vector.tensor_copy(xtr[:], xt[:])
                    ps = p0ps.tile([128, WCOL], dt.float32, tag="hps")
                    for kc in range(KC):
                        nc.tensor.matmul(
                            ps[:], xtr[:, kc * 128:(kc + 1) * 128],
                            W_sbr[:, kc * WCOL:(kc + 1) * WCOL],
                            start=(kc == 0), stop=(kc == KC - 1),
                        )
                    gsb = p0pool.tile([128, GROW], bf16, tag="gsb")
                    nc.vector.memset(gsb[:, F + 2 * H2:GROW], 0.0)
                    nc.vector.tensor_copy(gsb[:, 0:F], ps[:, 0:F])  # h -> bf16
                    nc.vector.tensor_copy(  # s_src raw f32 bits
                        gsb[:, F:F + H2].bitcast(dt.float32), ps[:, F:F + HEADS]
                    )
                    nc.vector.memset(gsb[:, F + H2:F + 2 * H2], 0.0)  # recip slot
                    nc.sync.dma_start(G_own[r * 128:(r + 1) * 128, :], gsb[:])
                    ssb = p0pool.tile([128, SROW], dt.float32, tag="ssb")
                    nc.vector.memset(ssb[:, H2:SROW], 0.0)
                    nc.vector.tensor_copy(ssb[:, 0:H2], ps[:, F:WCOL])
                    nc.vector.tensor_copy(
                        s_res[:, r * H2:(r + 1) * H2], ps[:, F:WCOL]
                    )
                    nc.sync.dma_start(S_own[r * 128:(r + 1) * 128, :], ssb[:])

            nc.gpsimd.collective_compute(
                "AllGather", Alu.bypass,
                ins=[S_own[:].opt()], outs=[S_full[:].opt()],
                replica_groups=[list(range(n_cores))],
            )

            # ---------------- phase 1: softmax denominators ----------------
            with (
                tc.tile_pool(name="p1idx", bufs=1) as p1i,
                tc.tile_pool(name="p1", bufs=4) as p1pool,
                tc.tile_pool(name="p1ps", bufs=2, space="PSUM") as p1ps,
            ):
                gidx_sb = p1i.tile([128, nblk * T1 * 8], dt.int16)
                nc.sync.dma_start(gidx_sb[:], p1_gidx[:])
                sidx_sb = p1i.tile([128, nblk * T1 * 8], dt.int16)
                nc.sync.dma_start(sidx_sb[:], p1_sidx[:])
                eq1_view = p1_eq[:].rearrange("(b t l) r -> b l t r", l=128, t=T1)

                for b in range(nblk):
                    co = b * T1 * 8
                    g1 = p1pool.tile([128, T1, SROW], dt.float32, tag="g1")
                    g2 = p1pool.tile([128, T1, SROW], dt.float32, tag="g2")
                    if t1_lo >= 2 and t1_hi >= 2:
                        la = (t1_lo + 1) // 2
                        ha = (t1_hi + 1) // 2
                        wq = [(T1 * (i + 1) // 4) - (T1 * i // 4) for i in range(4)]
                        calls = [
                            (g1, 0, la, S_full[0:HALF, :], 0, 0),
                            (g1, la, t1_lo - la, S_full[0:HALF, :], la, 1),
                            (g1, t1_lo, ha, S_full[HALF:npad, :], t1_lo, 2),
                            (g1, t1_lo + ha, t1_hi - ha, S_full[HALF:npad, :], t1_lo + ha, 3),
                        ]
                        wo = 0
                        for i in range(4):
                            calls.append((g2, wo, wq[i], S_own[b * 128:(b + 1) * 128, :], wo, i))
                            wo += wq[i]
                        for buf, t0, nt, src_ap, io, q in calls:
                            idxs = gidx_sb if buf is g1 else sidx_sb
                            nc.gpsimd.dma_gather(
                                buf[:, t0:t0 + nt, :], src_ap,
                                idxs[:, co + io * 8:co + (io + nt) * 8],
                                nt * 128, nt * 128, SROW,
                                single_packet=False, queue_num=q,
                            )
                    else:
                        nc.gpsimd.dma_gather(
                            g1[:, 0:t1_lo, :],
                            S_full[0:HALF, :] if npad > HALF else S_full[:, :],
                            gidx_sb[:, co:co + t1_lo * 8],
                            t1_lo * 128, t1_lo * 128, SROW,
                            single_packet=False, queue_num=0,
                        )
                        if t1_hi:
                            nc.gpsimd.dma_gather(
                                g1[:, t1_lo:T1, :], S_full[HALF:npad, :],
                                gidx_sb[:, co + t1_lo * 8:co + T1 * 8],
                                t1_hi * 128, t1_hi * 128, SROW,
                                single_packet=False, queue_num=0,
                            )
                        nc.gpsimd.dma_gather(
                            g2[:, :, :], S_own[b * 128:(b + 1) * 128, :],
                            sidx_sb[:, co:co + T1 * 8],
                            T1 * 128, T1 * 128, SROW,
                            single_packet=False, queue_num=0,
                        )
                    eq_sb = p1pool.tile([128, T1, 128], bf16, tag="eqs1")
                    nc.sync.dma_start(eq_sb[:], eq1_view[b])

                    # v = exp(leaky_relu(s_src + s_dst)) in bf16
                    z = p1pool.tile([128, T1 * HEADS], dt.float32, tag="z1")
                    z3 = z[:].rearrange("p (t h) -> p t h", h=HEADS)
                    nc.vector.tensor_tensor(
                        z3, g2[:, :, 0:HEADS], g1[:, :, HEADS:H2], op=Alu.add
                    )
                    nc.vector.scalar_tensor_tensor(
                        z[:], z[:], 0.2, z[:], op0=Alu.mult, op1=Alu.max
                    )
                    v = p1pool.tile([128, T1 * HEADS], bf16, tag="v1")
                    nc.scalar.activation(v[:], z[:], Act.Exp)

                    # sumexp^T [4, 128] += v_t^T @ EQ_t  (v stationary: 4 cols)
                    ps1 = p1ps.tile([4, 128], dt.float32, tag="ps1")
                    for t in range(T1):
                        nc.tensor.matmul(
                            ps1[:], v[:, t * HEADS:(t + 1) * HEADS],
                            eq_sb[:, t, :],
                            start=(t == 0), stop=(t == T1 - 1),
                        )
                    se_sb = p1pool.tile([4, 128], dt.float32, tag="se")
                    nc.vector.tensor_copy(se_sb[:], ps1[:])
                    ps_tr = p1ps.tile([128, 4], dt.float32, tag="ps1t")
                    nc.tensor.transpose(ps_tr[:], se_sb[:], ident[0:4, 0:4])
                    nc.vector.tensor_scalar_add(
                        rec_all[:, b * HEADS:(b + 1) * HEADS], ps_tr[:], 1e-10
                    )
                recd = cpool.tile([128, nblk * HEADS], dt.float32)
                nc.vector.reciprocal(recd[:], rec_all[:])
                for b in range(nblk):
                    nc.sync.dma_start(
                        G_own[b * 128:(b + 1) * 128,
                              F + H2:F + 2 * H2].bitcast(dt.float32),
                        recd[:, b * HEADS:(b + 1) * HEADS],
                    )

            nc.gpsimd.collective_compute(
                "AllGather", Alu.bypass,
                ins=[G_own[:].opt()], outs=[G_full[:].opt()],
                replica_groups=[list(range(n_cores))],
            )

            # ---------------- phase 2: aggregate messages ----------------
            with (
                tc.tile_pool(name="p2idx", bufs=1) as p2i,
                tc.tile_pool(name="p2", bufs=3) as p2pool,
                tc.tile_pool(name="p2ps", bufs=2, space="PSUM") as p2ps,
            ):
                gidx2_sb = p2i.tile([128, nblk * T2 * 8], dt.int16)
                nc.sync.dma_start(gidx2_sb[:], p2_gidx[:])
                sidx2_sb = p2i.tile([128, nblk * T2 * 8], dt.int16)
                nc.sync.dma_start(sidx2_sb[:], p2_sidx[:])
                eq2_view = p2_eq[:].rearrange("(b t l) r -> b l t r", l=128, t=T2)

                for b in range(nblk):
                    co = b * T2 * 8
                    g = p2pool.tile([128, T2, GROW], bf16, tag="g")
                    gs = p2pool.tile([128, T2, SROW], dt.float32, tag="gs")
                    if t2_lo >= 2 and t2_hi >= 2:
                        la = (t2_lo + 1) // 2
                        ha = (t2_hi + 1) // 2
                        wq = [(T2 * (i + 1) // 4) - (T2 * i // 4) for i in range(4)]
                        calls = [
                            (g, 0, la, G_full[0:HALF, :], 0, 0, GROW),
                            (g, la, t2_lo - la, G_full[0:HALF, :], la, 1, GROW),
                            (g, t2_lo, ha, G_full[HALF:npad, :], t2_lo, 2, GROW),
                            (g, t2_lo + ha, t2_hi - ha, G_full[HALF:npad, :], t2_lo + ha, 3, GROW),
                        ]
                        wo = 0
                        for i in range(4):
                            calls.append((gs, wo, wq[i], S_own[b * 128:(b + 1) * 128, :], wo, i, SROW))
                            wo += wq[i]
                        for buf, t0, nt, src_ap, io, q, esz in calls:
                            idxs = gidx2_sb if buf is g else sidx2_sb
                            nc.gpsimd.dma_gather(
                                buf[:, t0:t0 + nt, :], src_ap,
                                idxs[:, co + io * 8:co + (io + nt) * 8],
                                nt * 128, nt * 128, esz,
                                single_packet=False, queue_num=q,
                            )
                    else:
                        nc.gpsimd.dma_gather(
                            g[:, 0:t2_lo, :],
                            G_full[0:HALF, :] if npad > HALF else G_full[:, :],
                            gidx2_sb[:, co:co + t2_lo * 8],
                            t2_lo * 128, t2_lo * 128, GROW,
                            single_packet=False, queue_num=0,
                        )
                        if t2_hi:
                            nc.gpsimd.dma_gather(
                                g[:, t2_lo:T2, :], G_full[HALF:npad, :],
                                gidx2_sb[:, co + t2_lo * 8:co + T2 * 8],
                                t2_hi * 128, t2_hi * 128, GROW,
                                single_packet=False, queue_num=0,
                            )
                        nc.gpsimd.dma_gather(
                            gs[:, :, :], S_own[b * 128:(b + 1) * 128, :],
                            sidx2_sb[:, co:co + T2 * 8],
                            T2 * 128, T2 * 128, SROW,
                            single_packet=False, queue_num=0,
                        )
                    eq_sb = p2pool.tile([128, T2, 128], bf16, tag="eqs2")
                    nc.sync.dma_start(eq_sb[:], eq2_view[b])

                    # alpha = exp(leaky_relu(s_src + s_dst)) * recip  (f32)
                    al = p2pool.tile([128, T2 * HEADS], dt.float32, tag="al")
                    al3 = al[:].rearrange("p (t h) -> p t h", h=HEADS)
                    nc.vector.tensor_tensor(
                        al3,
                        g[:, :, F:F + H2].bitcast(dt.float32),
                        gs[:, :, HEADS:H2],
                        op=Alu.add,
                    )
                    nc.vector.scalar_tensor_tensor(
                        al[:], al[:], 0.2, al[:], op0=Alu.mult, op1=Alu.max
                    )
                    nc.scalar.activation(al[:], al[:], Act.Exp)
                    nc.vector.tensor_tensor(
                        al3, al3,
                        g[:, :, F + H2:F + 2 * H2].bitcast(dt.float32),
                        op=Alu.mult,
                    )

                    ps2 = p2ps.tile([128, F], dt.float32, tag="ps2")
                    for t in range(T2):
                        alpha_b = al[:, t * HEADS:(t + 1) * HEADS].unsqueeze(
                            2
                        ).broadcast_to([128, HEADS, OUT_DIM])
                        msg = p2pool.tile([128, F], bf16, tag="msg")
                        nc.vector.tensor_tensor(
                            msg[:].rearrange("p (h d) -> p h d", h=HEADS),
                            g[:, t, 0:F].rearrange("p (h d) -> p h d", h=HEADS),
                            alpha_b,
                            op=Alu.mult,
                        )
                        nc.tensor.matmul(
                            ps2[:], eq_sb[:, t, :], msg[:],
                            start=(t == 0), stop=(t == T2 - 1),
                        )
                    osb = p2pool.tile([128, F], dt.float32, tag="osb")
                    nc.vector.tensor_tensor(osb[:], ps2[:], bias_bc[:], op=Alu.add)
                    nc.sync.dma_start(out[b * 128:(b + 1) * 128, :], osb[:])

    nc.compile()
    return nc


def _gat_forward(x, edges, W, a, bias, n_nodes, n_cores, run_opts=None):
    npad = _ceil_div(n_nodes, n_cores * 128) * n_cores * 128
    rpc = npad // n_cores
    nblk = rpc // 128

    src = edges[:, 0].astype(np.int64)
    dst = edges[:, 1].astype(np.int64)
    t1_lo, t1_hi, p1_gidx, p1_sidx, p1_eq = _build_phase_meta(
        src, dst, src % 128, n_cores, nblk
    )
    t2_lo, t2_hi, p2_gidx, p2_sidx, p2_eq = _build_phase_meta(
        dst, src, dst % 128, n_cores, nblk
    )

    nc = _build_bass_program(npad, rpc, nblk, t1_lo, t1_hi, t2_lo, t2_hi, n_cores)

    x_pad = np.zeros((npad, IN_DIM), np.float32)
    x_pad[:n_nodes] = x
    xT = np.ascontiguousarray(x_pad.T)

    in_maps = []
    for c in range(n_cores):
        in_maps.append({
            "xT": np.ascontiguousarray(xT[:, c * rpc:(c + 1) * rpc]),
            "W": np.ascontiguousarray(W.astype(np.float32)),
            "a": np.ascontiguousarray(a.astype(np.float32)),
            "bias": np.ascontiguousarray(bias.astype(np.float32).reshape(1, F)),
            "p1_gidx": p1_gidx[c], "p1_sidx": p1_sidx[c], "p1_eq": p1_eq[c],
            "p2_gidx": p2_gidx[c], "p2_sidx": p2_sidx[c], "p2_eq": p2_eq[c],
        })

    from concourse.bass_utils import run_bass_kernel_spmd

    res = run_bass_kernel_spmd(
        nc, in_maps, core_ids=list(range(n_cores)), **(run_opts or {})
    )
    out = np.concatenate([r["out"] for r in res.results], axis=0)
    return out[:n_nodes], res


def kernel(x, edges, W, a, bias):
    x = np.asarray(x, np.float32)
    edges = np.asarray(edges)
    W = np.asarray(W, np.float32)
    a = np.asarray(a, np.float32)
    bias = np.asarray(bias, np.float32)
    out, _ = _gat_forward(x, edges, W, a, bias, N_NODES, N_CORES)
    return out

